# revision 1
# baseline (speedup 1.0000x reference)
"""BackflowNet GNN message-passing kernel for 8x Trainium2 NeuronCores.

Data-parallel over the walker axis B=128 -> 16 walkers per core, processed as
8 "pairs" (2 walkers block-diag-packed into the 128-partition dim).

Math restructuring (exact, host-side weight folding only):
  he0 = gelu(ein @ ew1 + eb1) @ ew2 + eb2           -> keep g_e = gelu(...)
  layer l: z = hv[:,i]@Wtop + he@Wbot + b1          (he = prev g @ w2 + b2 folded)
           g = gelu(z); he' = g @ w2 + b2
           m = gelu(he' @ e2v_w1 + e2v_b1)          (per-edge)
           hv += (sum_{i!=j} m_i) @ e2v_w2/(N-1) + e2v_b2   (sum moved before w2)
  head: dx = tanh(hv@hw1+hb1) @ (hw2*sp) + hb2*sp;  out = dx - mean_j dx
Per-edge tensors are feature-major [feat(part), e] with e = j*64 + i
(i = source = innermost so the aggregation is an innermost DVE reduce).

Matmuls run in float32r (FP22 in the PE array, 4x faster than fp32); every
tensor the PE consumes is declared float32r so the BIR verifier sees rounded
producers.
"""

import math
import os

import numpy as np

import concourse.bass as bass
import concourse.mybir as mybir
import concourse.tile as tile
import concourse.bass_utils as _bu
from concourse.bass_utils import run_bass_kernel_spmd

if os.environ.get("BACKFLOW_LDW_OPT", "0") == "1" and not getattr(_bu, "_ldw_patched", False):
    _bu._ldw_patched = True
    _orig_run = _bu.run_command

    def _run(cmd, cwd=None):
        if cmd and "walrus_driver" in cmd[0]:
            cmd = [c if c != "--enable-ldw-opt=false" else "--enable-ldw-opt=true"
                   for c in cmd]
        return _orig_run(cmd, cwd=cwd)

    _bu.run_command = _run

NCORES = 8
B, N, D = 128, 64, 2
H = 64
M = 64
BC = B // NCORES          # walkers per core
PAIRS = BC // 2           # walker pairs per core
E = N * N                 # edges (incl. diagonal) per walker
SUB = 512                 # matmul moving free dim
# gelu/psum blocks: uneven (3 ACT ops per pass instead of 4 cuts the
# per-op (N+352)/1.2 ns ACT overhead)
_B = int(os.environ.get("BACKFLOW_BLK", "1024"))
BLOCKS = ([(0, 1536), (1536, 1536), (3072, 1024)] if _B == 1536
          else [(0, 1024), (1024, 1024), (2048, 1024), (3072, 1024)])
PBLK = _B                 # psum tile width
F32 = mybir.dt.float32
F32R = mybir.dt.float32r
AF = mybir.ActivationFunctionType
AX = mybir.AxisListType

HU_DVE = os.environ.get("BACKFLOW_HU_DVE", "1") == "1"

_BUILT = {}


def _legalize_sync(bir_bytes):
    """Walrus on this toolchain encodes at most one semaphore wait per
    engine instruction (none on DMA queue entries). Tile attaches as many
    waits as deps require, so spill the surplus into standalone
    EventSemaphore instructions on the same engine, placed just before."""
    import json as _json

    d = _json.loads(bir_bytes)
    n = [0]

    def fix_block(bb):
        insts = bb.get("instructions")
        if not insts:
            return
        out = []
        for ins in insts:
            si = ins.get("sync_info")
            waits = (si or {}).get("on_wait") or []
            opc = ins.get("opcode", "")
            if opc == "EventSemaphore":
                allowed = 1
            elif opc.startswith("DMA") or ins.get("queue"):
                allowed = 0
            else:
                allowed = 1
            if len(waits) > allowed:
                keep, spill = waits[:allowed], waits[allowed:]
                for w in spill:
                    n[0] += 1
                    out.append({
                        "debug": ins.get("debug", 0),
                        "engine": ins["engine"],
                        "ins": [],
                        "outs": [],
                        "name": f"evw-{n[0]}",
                        "opcode": "EventSemaphore",
                        "sync_info": {"on_update": [], "on_wait": [w]},
                    })
                si["on_wait"] = keep
            out.append(ins)
        bb["instructions"] = out

    def walk(obj):
        if isinstance(obj, dict):
            if "instructions" in obj:
                fix_block(obj)
            else:
                for v in obj.values():
                    walk(v)
        elif isinstance(obj, list):
            for v in obj:
                walk(v)

    walk(d)
    return _json.dumps(d).encode()


def _build():
    nc = bass.Bass(
        "TRN2", target_bir_lowering=False, debug=False, enable_asserts=False
    )

    # All constants + per-core data in one packed dram tensor (one DMA).
    # Column map (fp32 elements; rows = partitions):
    #  0:128    wn1 [6,128]       128:256  wn2        256:384  wein [8,128]
    #  384:512  wc                512:640  wa0        640:768  wb1
    #  768:896  wa1               896:1024 wg0        1024:1152 wg1
    #  1152:1280 wt0              1280:1408 wt1       1408:1536 wh1
    #  1536:1540 wh2 [128,4]      1540:1604 eye [64,64]
    #  1604:1616 biases [128,12]  1616:1632 xp_lhs    1632:1760 xp_rhs
    #  1760:1776 xq_lhs           1776:1840 xq_rhs    1840:2352 nin [6,512]
    WCOLS = 2352
    wpack = nc.dram_tensor("wpack", [128, WCOLS], F32R, kind="ExternalInput").ap()
    out_dx = nc.dram_tensor("out_dx", [BC, N, D], F32, kind="ExternalOutput").ap()

    with tile.TileContext(nc) as tc:
        with (
            tc.tile_pool(name="consts", bufs=1) as consts,
            tc.tile_pool(name="feat", bufs=1) as feat,
            tc.tile_pool(name="big", bufs=7) as big,
            tc.tile_pool(name="gblk", bufs=8) as gblk,
            tc.tile_pool(name="small", bufs=24) as small,
            tc.tile_pool(name="swide", bufs=4) as swide,
            tc.tile_pool(name="pbig", bufs=(2 if _B == 1536 else 3),
                         space="PSUM") as pbig,
            tc.tile_pool(name="psmall", bufs=2, space="PSUM") as psmall,
        ):
            wp = consts.tile([128, WCOLS], F32R, name="wp", tag="wp")
            nc.sync.dma_start(out=wp, in_=wpack)
            wn1 = wp[0:6, 0:128]
            wn2 = wp[:, 128:256]
            wein = wp[0:8, 256:384]
            wc = wp[:, 384:512]
            wa0 = wp[:, 512:640]
            wb1 = wp[:, 640:768]
            wa1 = wp[:, 768:896]
            wg0 = wp[:, 896:1024]
            wg1 = wp[:, 1024:1152]
            wt0 = wp[:, 1152:1280]
            wt1 = wp[:, 1280:1408]
            wh1 = wp[:, 1408:1536]
            wh2 = wp[:, 1536:1540]
            eye_t = wp[0:64, 1540:1604]
            bia = wp[:, 1604:1616].bitcast(F32)
            xpl = wp[:, 1616:1632].bitcast(F32)
            xpr = wp[:, 1632:1760].bitcast(F32)
            xql = wp[:, 1760:1776].bitcast(F32)
            xqr = wp[:, 1776:1840].bitcast(F32)
            nin = wp[0:6, 1840:2352]
            eps_t = consts.tile([128, 1], F32, name="eps_t", tag="eps_t")
            nc.vector.memset(eps_t, 1e-12)

            # identity-broadcast rhs: rhs[k,(j,i)] = eye[k,i], j in 8-block
            eye_bc = eye_t.unsqueeze(1).broadcast_to([64, SUB // N, 64])


            import contextlib
            rep = int(os.environ.get("BACKFLOW_REPEAT", "1"))
            loop_cm = tc.For_i(0, rep, 1) if rep > 1 else contextlib.nullcontext()
            with loop_cm:
                # -------- phase 0: dr / r2 / rr in packed layouts ---------------
                # (w,d,jq2)-packed dr for the ein rows (contiguous per pair)
                dre_pk = feat.tile([128, 1024], F32R, name="dre_pk", tag="drepk")
                nc.vector.tensor_sub(
                    out=dre_pk.rearrange("p (j i) -> p j i", i=N),
                    in0=xql.unsqueeze(2).broadcast_to([128, 16, N]),
                    in1=xqr.unsqueeze(1).broadcast_to([128, 16, N]),
                )
                # (w,jq)-packed dr for r2/rr
                dr_pk = feat.tile([128, 1024], F32, name="dr_pk", tag="drpk")
                nc.vector.tensor_sub(
                    out=dr_pk.rearrange("p (d j i) -> p d j i", d=2, i=N),
                    in0=xpl.rearrange("p (d j) -> p d j", d=2)
                        .unsqueeze(3).broadcast_to([128, 2, 8, N]),
                    in1=xpr.rearrange("p (d i) -> p d i", d=2)
                        .unsqueeze(2).broadcast_to([128, 2, 8, N]),
                )
                sq_pk = feat.tile([128, 1024], F32, name="sq_pk", tag="sqpk")
                nc.vector.tensor_mul(out=sq_pk, in0=dr_pk, in1=dr_pk)
                sqv = sq_pk.rearrange("p (d f) -> p d f", d=2)
                r2_pk = feat.tile([128, 512], F32R, name="r2_pk", tag="r2pk")
                nc.vector.tensor_add(out=r2_pk, in0=sqv[:, 0, :], in1=sqv[:, 1, :])
                rr_pk = feat.tile([128, 512], F32R, name="rr_pk", tag="rrpk")
                sqrt_inst = nc.scalar.activation(
                    out=rr_pk, in_=r2_pk, func=AF.Sqrt, bias=eps_t, scale=1.0
                )

                # -------- batched node MLP (all 8 pairs) ------------------------
                zn = psmall.tile([128, 8 * N], F32, name="zn", tag="ps_s")
                zn_inst = nc.tensor.matmul(zn, wn1, nin)
                from concourse.tile_rust import add_dep_helper
                add_dep_helper(zn_inst.ins, sqrt_inst.ins,
                               reason="keep sqrt first in ACT stream (table set)")
                gn = swide.tile([128, 8 * N], F32R, name="gn", tag="sw")
                nc.scalar.activation(
                    out=gn, in_=zn, func=AF.Gelu, bias=bia[:, 0:1], scale=1.0
                )
                hv0p = psmall.tile([128, 8 * N], F32, name="hv0p", tag="ps_s")
                nc.tensor.matmul(hv0p, wn2, gn)
                hv0a = swide.tile([128, 8 * N], F32R, name="hv0a", tag="sw")
                nc.vector.tensor_scalar_add(out=hv0a, in0=hv0p, scalar1=bia[:, 1:2])

                # -------- software-pipelined per-pair stages --------------------
                st = [dict() for _ in range(PAIRS)]

                def edge_pass_blk(bl, dst, srcs, gelu_bias, hu_m=None):
                    lo0, blen = BLOCKS[bl]
                    ps = pbig.tile([128, PBLK], F32, name="ps", tag="ps_b")
                    for s in range(blen // SUB):
                        lo = lo0 + s * SUB
                        out_sl = ps[:, s * SUB : (s + 1) * SUB]
                        for k, (lhsT, rhs_fn) in enumerate(srcs):
                            nc.tensor.matmul(
                                out_sl,
                                lhsT,
                                rhs_fn(lo),
                                start=(k == 0),
                                stop=(k == len(srcs) - 1),
                            )
                    if hu_m is not None:
                        psv = ps[:, 0:blen].rearrange("p (j i) -> p j i", i=N)
                        nc.vector.tensor_add(
                            out=psv, in0=psv,
                            in1=hu_m.unsqueeze(1).broadcast_to(
                                [128, blen // N, N]
                            ),
                        )
                    nc.scalar.activation(
                        out=dst[:, 0:blen], in_=ps[:, 0:blen], func=AF.Gelu,
                        bias=gelu_bias, scale=1.0,
                    )

                def agg_layer(src_g, wagg, bias_col, acc_name):
                    acc = small.tile([128, N], F32, name=acc_name, tag="sm")
                    accd = small.tile([128, N], F32R, name=acc_name + "d", tag="sm")
                    for bl, (lo0, blen) in enumerate(BLOCKS):
                        gb = gblk.tile([128, PBLK], F32R, name="gb", tag="gb")
                        edge_pass_blk(
                            bl, gb,
                            [(wagg, lambda lo: src_g[:, lo : lo + SUB])],
                            bias_col,
                        )
                        jlo = lo0 // N
                        jb = blen // N
                        nc.vector.reduce_sum(
                            out=acc[:, jlo : jlo + jb],
                            in_=gb[:, 0:blen].rearrange("p (j i) -> p j i", i=N),
                            axis=AX.X,
                        )
                        diag = gb[:, jlo : jlo + (jb - 1) * (N + 1) + 1 : N + 1]
                        nc.vector.tensor_sub(
                            out=accd[:, jlo : jlo + jb],
                            in0=acc[:, jlo : jlo + jb],
                            in1=diag,
                        )
                    return accd

                def stage1(p):
                    s = st[p]
                    hv0 = hv0a[:, p * N : (p + 1) * N]
                    s["hv0"] = hv0
                    if HU_DVE:
                        # hu_m[m, i] = (Wtop0_bd.T @ hv0)[m, i] for DVE bcast-add
                        hu0p = psmall.tile([128, 64], F32, name="hu0p", tag="ps_s")
                        nc.tensor.matmul(hu0p, wt0, hv0)
                        hu0 = small.tile([128, 64], F32, name="hu0", tag="sm")
                        nc.vector.tensor_copy(out=hu0, in_=hu0p)
                    else:
                        hu0p = psmall.tile([64, 128], F32, name="hu0p", tag="ps_s")
                        nc.tensor.matmul(hu0p, hv0, wt0)
                        hu0 = small.tile([64, 128], F32R, name="hu0", tag="sm")
                        nc.vector.tensor_copy(out=hu0, in_=hu0p)
                    ein = big.tile([8, E], F32R, name="ein", tag="stream")
                    nc.gpsimd.dma_start(
                        out=ein[0:4, :], in_=dre_pk[16 * p : 16 * p + 16, :]
                    )
                    nc.gpsimd.dma_start(
                        out=ein[4:6, :], in_=r2_pk[16 * p : 16 * p + 16, :]
                    )
                    nc.gpsimd.dma_start(
                        out=ein[6:8, :], in_=rr_pk[16 * p : 16 * p + 16, :]
                    )
                    g1 = big.tile([128, E], F32R, name="g1", tag="stream")
                    s["g1"] = g1
                    for bl, (lo0, blen) in enumerate(BLOCKS):
                        geb = gblk.tile([128, PBLK], F32R, name="geb", tag="gb")
                        edge_pass_blk(
                            bl, geb,
                            [(wein, lambda lo: ein[:, lo : lo + SUB])],
                            bia[:, 2:3],
                        )
                        if HU_DVE:
                            edge_pass_blk(
                                bl, g1[:, lo0 : lo0 + blen],
                                [(wc, lambda lo: geb[:, lo - lo0 : lo - lo0 + SUB])],
                                bia[:, 3:4], hu_m=hu0,
                            )
                        else:
                            edge_pass_blk(
                                bl, g1[:, lo0 : lo0 + blen],
                                [
                                    (wc, lambda lo: geb[:, lo - lo0 : lo - lo0 + SUB]),
                                    (hu0, lambda lo: eye_bc),
                                ],
                                bia[:, 3:4],
                            )

                def stage2(p):
                    s = st[p]
                    accd0 = agg_layer(s["g1"], wa0, bia[:, 4:5], "acc0")
                    u0p = psmall.tile([128, N], F32, name="u0p", tag="ps_s")
                    nc.tensor.matmul(u0p, wg0, accd0)
                    hv1 = small.tile([128, N], F32R, name="hv1", tag="sm")
                    nc.vector.scalar_tensor_tensor(
                        out=hv1, in0=u0p, scalar=bia[:, 7:8], in1=s["hv0"],
                        op0=mybir.AluOpType.add, op1=mybir.AluOpType.add,
                    )
                    s["hv1"] = hv1
                    if HU_DVE:
                        hu1p = psmall.tile([128, 64], F32, name="hu1p", tag="ps_s")
                        nc.tensor.matmul(hu1p, wt1, hv1)
                        hu1 = small.tile([128, 64], F32, name="hu1", tag="sm")
                        nc.vector.tensor_copy(out=hu1, in_=hu1p)
                    else:
                        hu1p = psmall.tile([64, 128], F32, name="hu1p", tag="ps_s")
                        nc.tensor.matmul(hu1p, hv1, wt1)
                        hu1 = small.tile([64, 128], F32R, name="hu1", tag="sm")
                        nc.vector.tensor_copy(out=hu1, in_=hu1p)
                    s["hu1"] = hu1

                def stage3(p):
                    s = st[p]
                    g1 = s["g1"]
                    hu1 = s["hu1"]
                    g3 = big.tile([128, E], F32R, name="g3", tag="stream")
                    s["g3"] = g3
                    for bl, (lo0, blen) in enumerate(BLOCKS):
                        if HU_DVE:
                            edge_pass_blk(
                                bl, g3[:, lo0 : lo0 + blen],
                                [(wb1, lambda lo: g1[:, lo : lo + SUB])],
                                bia[:, 5:6], hu_m=hu1,
                            )
                        else:
                            edge_pass_blk(
                                bl, g3[:, lo0 : lo0 + blen],
                                [
                                    (wb1, lambda lo: g1[:, lo : lo + SUB]),
                                    (hu1, lambda lo: eye_bc),
                                ],
                                bia[:, 5:6],
                            )

                def stage4(p):
                    s = st[p]
                    accd1 = agg_layer(s["g3"], wa1, bia[:, 6:7], "acc1")
                    u1p = psmall.tile([128, N], F32, name="u1p", tag="ps_s")
                    nc.tensor.matmul(u1p, wg1, accd1)
                    hv2 = small.tile([128, N], F32R, name="hv2", tag="sm")
                    nc.vector.scalar_tensor_tensor(
                        out=hv2, in0=u1p, scalar=bia[:, 8:9], in1=s["hv1"],
                        op0=mybir.AluOpType.add, op1=mybir.AluOpType.add,
                    )
                    thp = psmall.tile([128, N], F32, name="thp", tag="ps_s")
                    nc.tensor.matmul(thp, wh1, hv2)
                    th = small.tile([128, N], F32R, name="th", tag="sm")
                    nc.scalar.activation(
                        out=th, in_=thp, func=AF.Tanh, bias=bia[:, 9:10], scale=1.0
                    )
                    dxp = psmall.tile([4, N], F32, name="dxp", tag="ps_s")
                    nc.tensor.matmul(dxp, wh2, th)
                    dx = small.tile([4, N], F32, name="dx", tag="sm")
                    nc.vector.tensor_scalar_add(out=dx, in0=dxp, scalar1=bia[0:4, 10:11])
                    msum = small.tile([4, 1], F32, name="msum", tag="sm1")
                    nc.vector.reduce_sum(out=msum, in_=dx, axis=AX.X)
                    negm = small.tile([4, 1], F32, name="negm", tag="sm1")
                    nc.vector.tensor_scalar_mul(out=negm, in0=msum, scalar1=-1.0 / N)
                    dxf = small.tile([4, N], F32, name="dxf", tag="sm")
                    nc.vector.tensor_scalar_add(out=dxf, in0=dx, scalar1=negm)
                    nc.sync.dma_start(
                        out=out_dx[2 * p].transpose([1, 0]), in_=dxf[0:2, :]
                    )
                    nc.sync.dma_start(
                        out=out_dx[2 * p + 1].transpose([1, 0]), in_=dxf[2:4, :]
                    )

                stages = [stage1, stage2, stage3, stage4]
                order = [int(c) for c in os.environ.get("BACKFLOW_ORDER", "0132")]
                for t in range(PAIRS + len(stages) - 1):
                    for si in order:
                        p = t - si
                        if 0 <= p < PAIRS:
                            stages[si](p)

    patched = _legalize_sync(nc.to_json_bytes())
    nc.to_json_bytes = lambda: patched
    return nc


def _prep_weights(inputs):
    f8 = np.float64
    g = {k: np.asarray(v, dtype=f8) for k, v in inputs.items()}
    inv = 1.0 / (N - 1)

    wtop0 = g["v2e_w1"][0][:H]
    wbot0 = g["v2e_w1"][0][H:]
    wtop1 = g["v2e_w1"][1][:H]
    wbot1 = g["v2e_w1"][1][H:]

    w_c = g["edge_w2"] @ wbot0
    b_p2 = g["edge_b2"] @ wbot0 + g["v2e_b1"][0]
    w_a0 = g["v2e_w2"][0] @ g["e2v_w1"][0]
    b_p3 = g["v2e_b2"][0] @ g["e2v_w1"][0] + g["e2v_b1"][0]
    w_b1 = g["v2e_w2"][0] @ wbot1
    b_p4 = g["v2e_b2"][0] @ wbot1 + g["v2e_b1"][1]
    w_a1 = g["v2e_w2"][1] @ g["e2v_w1"][1]
    b_p5 = g["v2e_b2"][1] @ g["e2v_w1"][1] + g["e2v_b1"][1]
    w_g0 = g["e2v_w2"][0] * inv
    w_g1 = g["e2v_w2"][1] * inv
    sp = float(np.log1p(np.exp(g["scale"][0])))
    w_h2 = g["head_w2"] * sp
    b_h2 = g["head_b2"] * sp

    def bd(w):  # [64,64] -> [128,128] block-diag
        o = np.zeros((128, 128), f8)
        o[:64, :64] = w
        o[64:, 64:] = w
        return o

    def dup(b):  # [64] -> [128]
        return np.concatenate([b, b])

    ws = {}
    wn1 = np.zeros((6, 128), f8)
    wn1[0:3, 0:64] = g["node_w1"]
    wn1[3:6, 64:128] = g["node_w1"]
    ws["w_node1"] = wn1
    ws["w_node2"] = bd(g["node_w2"])
    e1 = g["edge_w1"]
    wein = np.zeros((8, 128), f8)
    wein[0, 0:64] = e1[0]     # dr0 walker a
    wein[1, 0:64] = e1[1]     # dr1 walker a
    wein[2, 64:128] = e1[0]   # dr0 walker b
    wein[3, 64:128] = e1[1]   # dr1 walker b
    wein[4, 0:64] = e1[3]     # r2 walker a
    wein[5, 64:128] = e1[3]   # r2 walker b
    wein[6, 0:64] = e1[2]     # rr walker a
    wein[7, 64:128] = e1[2]   # rr walker b
    ws["w_ein"] = wein
    ws["w_c"] = bd(w_c)
    ws["w_a0"] = bd(w_a0)
    ws["w_b1"] = bd(w_b1)
    ws["w_a1"] = bd(w_a1)
    ws["w_g0"] = bd(w_g0)
    ws["w_g1"] = bd(w_g1)
    ws["w_top0"] = bd(wtop0)
    ws["w_top1"] = bd(wtop1)
    ws["w_h1"] = bd(g["head_w1"])
    wh2 = np.zeros((128, 4), f8)
    wh2[0:64, 0:2] = w_h2
    wh2[64:128, 2:4] = w_h2
    ws["w_h2"] = wh2
    ws["eye64"] = np.eye(64, dtype=f8)

    bias = np.zeros((128, 12), f8)
    bias[:, 0] = dup(g["node_b1"])
    bias[:, 1] = dup(g["node_b2"])
    bias[:, 2] = dup(g["edge_b1"])
    bias[:, 3] = dup(b_p2)
    bias[:, 4] = dup(b_p3)
    bias[:, 5] = dup(b_p4)
    bias[:, 6] = dup(b_p5)
    bias[:, 7] = dup(g["e2v_b2"][0])
    bias[:, 8] = dup(g["e2v_b2"][1])
    bias[:, 9] = dup(g["head_b1"])
    bias[0:4, 10] = [b_h2[0], b_h2[1], b_h2[0], b_h2[1]]
    ws["biases"] = bias
    return {k: np.ascontiguousarray(v, dtype=np.float32) for k, v in ws.items()}


def _pack_consts(ws, xt, st_):
    """Assemble the [128, 2352] wpack array (see _build column map)."""
    wp = np.zeros((128, 2352), np.float32)

    def put(col, arr, rows=128):
        a = np.asarray(arr, np.float32)
        wp[: a.shape[0], col : col + a.shape[1]] = a

    put(0, ws["w_node1"])
    put(128, ws["w_node2"])
    put(256, ws["w_ein"])
    put(384, ws["w_c"])
    put(512, ws["w_a0"])
    put(640, ws["w_b1"])
    put(768, ws["w_a1"])
    put(896, ws["w_g0"])
    put(1024, ws["w_g1"])
    put(1152, ws["w_top0"])
    put(1280, ws["w_top1"])
    put(1408, ws["w_h1"])
    put(1536, ws["w_h2"])
    put(1540, ws["eye64"])
    put(1604, ws["biases"])
    BCl, Nl = xt.shape[0], xt.shape[2]
    put(1616, xt.reshape(BCl, 2, 8, 8).transpose(0, 2, 1, 3).reshape(128, 16))
    put(1632, np.repeat(xt.reshape(BCl, 1, 2 * Nl), 8, axis=1).reshape(128, 2 * Nl))
    put(1760, xt.reshape(128, 16))
    put(1776, np.repeat(xt.reshape(BCl * 2, 1, Nl), 4, axis=1).reshape(128, Nl))
    nin = np.concatenate([xt, st_], axis=1).reshape(PAIRS, 6, Nl)
    put(1840, nin.transpose(1, 0, 2).reshape(6, 8 * Nl))
    return wp


def kernel(**inputs) -> np.ndarray:
    x = np.asarray(inputs["x"], dtype=np.float32)       # [B, N, D]
    spin = np.asarray(inputs["spin"], dtype=np.float32) # [B, N, 1]
    ws = _prep_weights(inputs)

    if "nc" not in _BUILT:
        _BUILT["nc"] = _build()
    nc = _BUILT["nc"]

    in_maps = []
    for c in range(NCORES):
        xc = x[c * BC : (c + 1) * BC]                     # [16, N, 2]
        sc = spin[c * BC : (c + 1) * BC]                  # [16, N, 1]
        xt = np.ascontiguousarray(xc.transpose(0, 2, 1))  # [16, 2, N]
        st = np.ascontiguousarray(sc.transpose(0, 2, 1))  # [16, 1, N]
        in_maps.append({"wpack": _pack_consts(ws, xt, st)})

    res = run_bass_kernel_spmd(
        nc,
        in_maps,
        core_ids=list(range(NCORES)),
        trace=os.environ.get("BACKFLOW_TRACE", "0") == "1",
    )
    kernel.last_results = res
    out = np.concatenate([r["out_dx"] for r in res.results], axis=0)
    return out.astype(np.float32)



# revision 15
# speedup vs baseline: 1.2115x; 1.2115x over previous
"""BackflowNet GNN message-passing kernel for 8x Trainium2 NeuronCores.

Data-parallel over the walker axis B=128 -> 16 walkers per core, processed as
8 "pairs" (2 walkers block-diag-packed into the 128-partition dim).

Math restructuring (exact, host-side weight folding only):
  he0 = gelu(ein @ ew1 + eb1) @ ew2 + eb2           -> keep g_e = gelu(...)
  layer l: z = hv[:,i]@Wtop + he@Wbot + b1          (he = prev g @ w2 + b2 folded)
           g = gelu(z); he' = g @ w2 + b2
           m = gelu(he' @ e2v_w1 + e2v_b1)          (per-edge)
           hv += (sum_{i!=j} m_i) @ e2v_w2/(N-1) + e2v_b2   (sum moved before w2)
  head: dx = tanh(hv@hw1+hb1) @ (hw2*sp) + hb2*sp;  out = dx - mean_j dx
Per-edge tensors are feature-major [feat(part), e] with e = j*64 + i
(i = source = innermost so the aggregation is an innermost DVE reduce).

Matmuls run in float32r (FP22 in the PE array, 4x faster than fp32); every
tensor the PE consumes is declared float32r so the BIR verifier sees rounded
producers.
"""

import math
import os

import numpy as np

import concourse.bass as bass
import concourse.mybir as mybir
import concourse.tile as tile
import concourse.bass_utils as _bu
from concourse.bass_utils import run_bass_kernel_spmd

if os.environ.get("BACKFLOW_LDW_OPT", "0") == "1" and not getattr(_bu, "_ldw_patched", False):
    _bu._ldw_patched = True
    _orig_run = _bu.run_command

    def _run(cmd, cwd=None):
        if cmd and "walrus_driver" in cmd[0]:
            cmd = [c if c != "--enable-ldw-opt=false" else "--enable-ldw-opt=true"
                   for c in cmd]
        return _orig_run(cmd, cwd=cwd)

    _bu.run_command = _run

NCORES = 8
B, N, D = 128, 64, 2
H = 64
M = 64
BC = B // NCORES          # walkers per core
PAIRS = BC // 2           # walker pairs per core
E = N * N                 # edges (incl. diagonal) per walker
SUB = 512                 # matmul moving free dim
# gelu/psum blocks: uneven (3 ACT ops per pass instead of 4 cuts the
# per-op (N+352)/1.2 ns ACT overhead)
_B = int(os.environ.get("BACKFLOW_BLK", "1536"))
BLOCKS = ([(0, 1536), (1536, 1536), (3072, 1024)] if _B == 1536
          else [(0, 1024), (1024, 1024), (2048, 1024), (3072, 1024)])
PBLK = _B                 # psum tile width
F32 = mybir.dt.float32
F32R = mybir.dt.float32r
BF16 = mybir.dt.bfloat16
AF = mybir.ActivationFunctionType
AX = mybir.AxisListType

HU_DVE = os.environ.get("BACKFLOW_HU_DVE", "1") == "1"
# bf16 edge-feature tensors: gelu outputs quantized to bf16 (ACT cost is
# dtype-independent; DVE tensor_tensor gets the 2x_1p mode, matmul rhs
# bf16 is 1 cyc/row). The i-aggregation becomes 2 bf16 fold-adds (2x)
# + a short 1x reduce instead of one full-width 1x reduce.
EDGE_BF16 = os.environ.get("BACKFLOW_BF16", "1") == "1"
EDT = BF16 if EDGE_BF16 else F32R
# PE HAM warmup: ~8 back-to-back matmuls (~5us cold) fill the 4096-cycle
# activity window so the PE clock ungates to 2.4 GHz before real work.
NWARM = int(os.environ.get("BACKFLOW_WARM", "8"))

_BUILT = {}


def _legalize_sync(bir_bytes):
    """Walrus on this toolchain encodes at most one semaphore wait per
    engine instruction (none on DMA queue entries). Tile attaches as many
    waits as deps require, so spill the surplus into standalone
    EventSemaphore instructions on the same engine, placed just before."""
    import json as _json

    d = _json.loads(bir_bytes)
    n = [0]

    def fix_block(bb):
        insts = bb.get("instructions")
        if not insts:
            return
        out = []
        for ins in insts:
            si = ins.get("sync_info")
            waits = (si or {}).get("on_wait") or []
            opc = ins.get("opcode", "")
            if opc == "EventSemaphore":
                allowed = 1
            elif opc.startswith("DMA") or ins.get("queue"):
                allowed = 0
            else:
                allowed = 1
            if len(waits) > allowed:
                keep, spill = waits[:allowed], waits[allowed:]
                for w in spill:
                    n[0] += 1
                    out.append({
                        "debug": ins.get("debug", 0),
                        "engine": ins["engine"],
                        "ins": [],
                        "outs": [],
                        "name": f"evw-{n[0]}",
                        "opcode": "EventSemaphore",
                        "sync_info": {"on_update": [], "on_wait": [w]},
                    })
                si["on_wait"] = keep
            out.append(ins)
        bb["instructions"] = out

    def walk(obj):
        if isinstance(obj, dict):
            if "instructions" in obj:
                fix_block(obj)
            else:
                for v in obj.values():
                    walk(v)
        elif isinstance(obj, list):
            for v in obj:
                walk(v)

    walk(d)
    return _json.dumps(d).encode()


def _build():
    nc = bass.Bass(
        "TRN2", target_bir_lowering=False, debug=False, enable_asserts=False
    )

    # All constants + per-core data in one packed dram tensor (one DMA).
    # Column map (fp32 elements; rows = partitions):
    #  0:128    wn1 [6,128]       128:256  wn2        256:384  wein [8,128]
    #  384:512  wc                512:640  wa0        640:768  wb1
    #  768:896  wa1               896:1024 wg0        1024:1152 wg1
    #  1152:1280 wt0              1280:1408 wt1       1408:1536 wh1
    #  1536:1540 wh2 [128,4]      1540:1604 eye [64,64]
    #  1604:1616 biases [128,12]  1616:1632 xp_lhs    1632:1760 xp_rhs
    #  1760:1776 xq_lhs           1776:1840 xq_rhs    1840:2352 nin [6,512]
    #  2352:2608 bf16-packed wc/wa0/wb1/wa1 (64 fp32 cols = 128 bf16 each)
    WCOLS = 2608
    wpack = nc.dram_tensor("wpack", [128, WCOLS], F32R, kind="ExternalInput").ap()
    out_dx = nc.dram_tensor("out_dx", [BC, N, D], F32, kind="ExternalOutput").ap()

    with tile.TileContext(nc) as tc:
        with (
            tc.tile_pool(name="consts", bufs=1) as consts,
            tc.tile_pool(name="feat", bufs=1) as feat,
            tc.tile_pool(name="big", bufs=7) as big,
            tc.tile_pool(name="gblk", bufs=8) as gblk,
            tc.tile_pool(name="fold", bufs=4) as fold,
            tc.tile_pool(name="small", bufs=24) as small,
            tc.tile_pool(name="swide", bufs=4) as swide,
            tc.tile_pool(name="pbig", bufs=(2 if _B == 1536 else 3),
                         space="PSUM") as pbig,
            tc.tile_pool(name="psmall", bufs=2, space="PSUM") as psmall,
        ):
            wp = consts.tile([128, WCOLS], F32R, name="wp", tag="wp")
            nc.sync.dma_start(out=wp, in_=wpack)
            wn1 = wp[0:6, 0:128]
            wn2 = wp[:, 128:256]
            wein = wp[0:8, 256:384]
            wc = wp[:, 384:512]
            wa0 = wp[:, 512:640]
            wb1 = wp[:, 640:768]
            wa1 = wp[:, 768:896]
            wg0 = wp[:, 896:1024]
            wg1 = wp[:, 1024:1152]
            wt0 = wp[:, 1152:1280]
            wt1 = wp[:, 1280:1408]
            wh1 = wp[:, 1408:1536]
            wh2 = wp[:, 1536:1540]
            eye_t = wp[0:64, 1540:1604]
            wc_bf = wp[:, 2352:2416].bitcast(BF16)
            wa0_bf = wp[:, 2416:2480].bitcast(BF16)
            wb1_bf = wp[:, 2480:2544].bitcast(BF16)
            wa1_bf = wp[:, 2544:2608].bitcast(BF16)
            if EDGE_BF16:
                wc_e, wa0_e, wb1_e, wa1_e = wc_bf, wa0_bf, wb1_bf, wa1_bf
            else:
                wc_e, wa0_e, wb1_e, wa1_e = wc, wa0, wb1, wa1
            bia = wp[:, 1604:1616].bitcast(F32)
            xpl = wp[:, 1616:1632].bitcast(F32)
            xpr = wp[:, 1632:1760].bitcast(F32)
            xql = wp[:, 1760:1776].bitcast(F32)
            xqr = wp[:, 1776:1840].bitcast(F32)
            nin = wp[0:6, 1840:2352]
            eps_t = consts.tile([128, 1], F32, name="eps_t", tag="eps_t")
            nc.vector.memset(eps_t, 1e-12)

            if NWARM:
                warm_w = consts.tile([128, 128], F32R, name="warm_w", tag="warm")
                nc.vector.memset(warm_w.bitcast(F32), 0.0)
                wmp = psmall.tile([128, 512], F32, name="wmp", tag="ps_s")
                warm_rhs = warm_w.unsqueeze(1).broadcast_to([128, 4, 128])
                for k in range(NWARM):
                    nc.tensor.matmul(
                        wmp, warm_w, warm_rhs,
                        start=(k == 0), stop=(k == NWARM - 1),
                    )

            # identity-broadcast rhs: rhs[k,(j,i)] = eye[k,i], j in 8-block
            eye_bc = eye_t.unsqueeze(1).broadcast_to([64, SUB // N, 64])


            import contextlib
            rep = int(os.environ.get("BACKFLOW_REPEAT", "1"))
            loop_cm = tc.For_i(0, rep, 1) if rep > 1 else contextlib.nullcontext()
            with loop_cm:
                # -------- phase 0: dr / r2 / rr in packed layouts ---------------
                # (w,d,jq2)-packed dr for the ein rows (contiguous per pair)
                dre_pk = feat.tile([128, 1024], F32R, name="dre_pk", tag="drepk")
                nc.vector.tensor_sub(
                    out=dre_pk.rearrange("p (j i) -> p j i", i=N),
                    in0=xql.unsqueeze(2).broadcast_to([128, 16, N]),
                    in1=xqr.unsqueeze(1).broadcast_to([128, 16, N]),
                )
                # (w,jq)-packed dr for r2/rr
                dr_pk = feat.tile([128, 1024], F32, name="dr_pk", tag="drpk")
                nc.vector.tensor_sub(
                    out=dr_pk.rearrange("p (d j i) -> p d j i", d=2, i=N),
                    in0=xpl.rearrange("p (d j) -> p d j", d=2)
                        .unsqueeze(3).broadcast_to([128, 2, 8, N]),
                    in1=xpr.rearrange("p (d i) -> p d i", d=2)
                        .unsqueeze(2).broadcast_to([128, 2, 8, N]),
                )
                sq_pk = feat.tile([128, 1024], F32, name="sq_pk", tag="sqpk")
                nc.vector.tensor_mul(out=sq_pk, in0=dr_pk, in1=dr_pk)
                sqv = sq_pk.rearrange("p (d f) -> p d f", d=2)
                r2_pk = feat.tile([128, 512], F32R, name="r2_pk", tag="r2pk")
                nc.vector.tensor_add(out=r2_pk, in0=sqv[:, 0, :], in1=sqv[:, 1, :])
                rr_pk = feat.tile([128, 512], F32R, name="rr_pk", tag="rrpk")
                sqrt_inst = nc.scalar.activation(
                    out=rr_pk, in_=r2_pk, func=AF.Sqrt, bias=eps_t, scale=1.0
                )

                # -------- batched node MLP (all 8 pairs) ------------------------
                zn = psmall.tile([128, 8 * N], F32, name="zn", tag="ps_s")
                zn_inst = nc.tensor.matmul(zn, wn1, nin)
                from concourse.tile_rust import add_dep_helper
                add_dep_helper(zn_inst.ins, sqrt_inst.ins,
                               reason="keep sqrt first in ACT stream (table set)")
                gn = swide.tile([128, 8 * N], F32R, name="gn", tag="sw")
                nc.scalar.activation(
                    out=gn, in_=zn, func=AF.Gelu, bias=bia[:, 0:1], scale=1.0
                )
                hv0p = psmall.tile([128, 8 * N], F32, name="hv0p", tag="ps_s")
                nc.tensor.matmul(hv0p, wn2, gn)
                hv0a = swide.tile([128, 8 * N], F32R, name="hv0a", tag="sw")
                nc.vector.tensor_scalar_add(out=hv0a, in0=hv0p, scalar1=bia[:, 1:2])

                # -------- software-pipelined per-pair stages --------------------
                st = [dict() for _ in range(PAIRS)]

                def edge_pass_blk(bl, dst, srcs, gelu_bias, hu_m=None):
                    lo0, blen = BLOCKS[bl]
                    ps = pbig.tile([128, PBLK], F32, name="ps", tag="ps_b")
                    for s in range(blen // SUB):
                        lo = lo0 + s * SUB
                        out_sl = ps[:, s * SUB : (s + 1) * SUB]
                        for k, (lhsT, rhs_fn) in enumerate(srcs):
                            nc.tensor.matmul(
                                out_sl,
                                lhsT,
                                rhs_fn(lo),
                                start=(k == 0),
                                stop=(k == len(srcs) - 1),
                            )
                    if hu_m is not None:
                        psv = ps[:, 0:blen].rearrange("p (j i) -> p j i", i=N)
                        nc.vector.tensor_add(
                            out=psv, in0=psv,
                            in1=hu_m.unsqueeze(1).broadcast_to(
                                [128, blen // N, N]
                            ),
                        )
                    nc.scalar.activation(
                        out=dst[:, 0:blen], in_=ps[:, 0:blen], func=AF.Gelu,
                        bias=gelu_bias, scale=1.0,
                    )

                def agg_layer(src_g, wagg, bias_col, acc_name):
                    acc = small.tile([128, N], F32, name=acc_name, tag="sm")
                    accd = small.tile([128, N], F32R, name=acc_name + "d", tag="sm")
                    mjb = PBLK // N
                    for bl, (lo0, blen) in enumerate(BLOCKS):
                        gb = gblk.tile([128, PBLK], EDT, name="gb", tag="gb")
                        edge_pass_blk(
                            bl, gb,
                            [(wagg, lambda lo: src_g[:, lo : lo + SUB])],
                            bias_col,
                        )
                        jlo = lo0 // N
                        jb = blen // N
                        gv = gb[:, 0:blen].rearrange("p (j i) -> p j i", i=N)
                        if EDGE_BF16:
                            f1 = fold.tile([128, mjb, 32], BF16, name="f1",
                                           tag="f1")
                            nc.vector.tensor_add(
                                out=f1[:, 0:jb, :], in0=gv[:, :, 0:32],
                                in1=gv[:, :, 32:64],
                            )
                            f2 = fold.tile([128, mjb, 16], BF16, name="f2",
                                           tag="f2")
                            nc.vector.tensor_add(
                                out=f2[:, 0:jb, :], in0=f1[:, 0:jb, 0:16],
                                in1=f1[:, 0:jb, 16:32],
                            )
                            nc.vector.reduce_sum(
                                out=acc[:, jlo : jlo + jb], in_=f2[:, 0:jb, :],
                                axis=AX.X,
                            )
                        else:
                            nc.vector.reduce_sum(
                                out=acc[:, jlo : jlo + jb], in_=gv, axis=AX.X,
                            )
                        diag = gb[:, jlo : jlo + (jb - 1) * (N + 1) + 1 : N + 1]
                        nc.vector.tensor_sub(
                            out=accd[:, jlo : jlo + jb],
                            in0=acc[:, jlo : jlo + jb],
                            in1=diag,
                        )
                    return accd

                def stage1(p):
                    s = st[p]
                    hv0 = hv0a[:, p * N : (p + 1) * N]
                    s["hv0"] = hv0
                    if HU_DVE:
                        # hu_m[m, i] = (Wtop0_bd.T @ hv0)[m, i] for DVE bcast-add
                        hu0p = psmall.tile([128, 64], F32, name="hu0p", tag="ps_s")
                        nc.tensor.matmul(hu0p, wt0, hv0)
                        hu0 = small.tile([128, 64], F32, name="hu0", tag="sm")
                        nc.vector.tensor_copy(out=hu0, in_=hu0p)
                    else:
                        hu0p = psmall.tile([64, 128], F32, name="hu0p", tag="ps_s")
                        nc.tensor.matmul(hu0p, hv0, wt0)
                        hu0 = small.tile([64, 128], F32R, name="hu0", tag="sm")
                        nc.vector.tensor_copy(out=hu0, in_=hu0p)
                    if EDGE_BF16:
                        ein = big.tile([8, E], F32R, name="ein", tag="ein",
                                       bufs=3)
                    else:
                        ein = big.tile([8, E], F32R, name="ein", tag="stream")
                    nc.gpsimd.dma_start(
                        out=ein[0:4, :], in_=dre_pk[16 * p : 16 * p + 16, :]
                    )
                    nc.gpsimd.dma_start(
                        out=ein[4:6, :], in_=r2_pk[16 * p : 16 * p + 16, :]
                    )
                    nc.gpsimd.dma_start(
                        out=ein[6:8, :], in_=rr_pk[16 * p : 16 * p + 16, :]
                    )
                    g1 = big.tile([128, E], EDT, name="g1", tag="stream")
                    s["g1"] = g1
                    for bl, (lo0, blen) in enumerate(BLOCKS):
                        geb = gblk.tile([128, PBLK], EDT, name="geb", tag="gb")
                        edge_pass_blk(
                            bl, geb,
                            [(wein, lambda lo: ein[:, lo : lo + SUB])],
                            bia[:, 2:3],
                        )
                        if HU_DVE:
                            edge_pass_blk(
                                bl, g1[:, lo0 : lo0 + blen],
                                [(wc_e, lambda lo: geb[:, lo - lo0 : lo - lo0 + SUB])],
                                bia[:, 3:4], hu_m=hu0,
                            )
                        else:
                            edge_pass_blk(
                                bl, g1[:, lo0 : lo0 + blen],
                                [
                                    (wc_e, lambda lo: geb[:, lo - lo0 : lo - lo0 + SUB]),
                                    (hu0, lambda lo: eye_bc),
                                ],
                                bia[:, 3:4],
                            )

                def stage2(p):
                    s = st[p]
                    accd0 = agg_layer(s["g1"], wa0_e, bia[:, 4:5], "acc0")
                    u0p = psmall.tile([128, N], F32, name="u0p", tag="ps_s")
                    nc.tensor.matmul(u0p, wg0, accd0)
                    hv1 = small.tile([128, N], F32R, name="hv1", tag="sm")
                    nc.vector.scalar_tensor_tensor(
                        out=hv1, in0=u0p, scalar=bia[:, 7:8], in1=s["hv0"],
                        op0=mybir.AluOpType.add, op1=mybir.AluOpType.add,
                    )
                    s["hv1"] = hv1
                    if HU_DVE:
                        hu1p = psmall.tile([128, 64], F32, name="hu1p", tag="ps_s")
                        nc.tensor.matmul(hu1p, wt1, hv1)
                        hu1 = small.tile([128, 64], F32, name="hu1", tag="sm")
                        nc.vector.tensor_copy(out=hu1, in_=hu1p)
                    else:
                        hu1p = psmall.tile([64, 128], F32, name="hu1p", tag="ps_s")
                        nc.tensor.matmul(hu1p, hv1, wt1)
                        hu1 = small.tile([64, 128], F32R, name="hu1", tag="sm")
                        nc.vector.tensor_copy(out=hu1, in_=hu1p)
                    s["hu1"] = hu1

                def stage3(p):
                    s = st[p]
                    g1 = s["g1"]
                    hu1 = s["hu1"]
                    g3 = big.tile([128, E], EDT, name="g3", tag="stream")
                    s["g3"] = g3
                    for bl, (lo0, blen) in enumerate(BLOCKS):
                        if HU_DVE:
                            edge_pass_blk(
                                bl, g3[:, lo0 : lo0 + blen],
                                [(wb1_e, lambda lo: g1[:, lo : lo + SUB])],
                                bia[:, 5:6], hu_m=hu1,
                            )
                        else:
                            edge_pass_blk(
                                bl, g3[:, lo0 : lo0 + blen],
                                [
                                    (wb1_e, lambda lo: g1[:, lo : lo + SUB]),
                                    (hu1, lambda lo: eye_bc),
                                ],
                                bia[:, 5:6],
                            )

                def stage4(p):
                    s = st[p]
                    accd1 = agg_layer(s["g3"], wa1_e, bia[:, 6:7], "acc1")
                    u1p = psmall.tile([128, N], F32, name="u1p", tag="ps_s")
                    nc.tensor.matmul(u1p, wg1, accd1)
                    hv2 = small.tile([128, N], F32R, name="hv2", tag="sm")
                    nc.vector.scalar_tensor_tensor(
                        out=hv2, in0=u1p, scalar=bia[:, 8:9], in1=s["hv1"],
                        op0=mybir.AluOpType.add, op1=mybir.AluOpType.add,
                    )
                    thp = psmall.tile([128, N], F32, name="thp", tag="ps_s")
                    nc.tensor.matmul(thp, wh1, hv2)
                    th = small.tile([128, N], F32R, name="th", tag="sm")
                    nc.scalar.activation(
                        out=th, in_=thp, func=AF.Tanh, bias=bia[:, 9:10], scale=1.0
                    )
                    dxp = psmall.tile([4, N], F32, name="dxp", tag="ps_s")
                    nc.tensor.matmul(dxp, wh2, th)
                    dx = small.tile([4, N], F32, name="dx", tag="sm")
                    nc.vector.tensor_scalar_add(out=dx, in0=dxp, scalar1=bia[0:4, 10:11])
                    msum = small.tile([4, 1], F32, name="msum", tag="sm1")
                    nc.vector.reduce_sum(out=msum, in_=dx, axis=AX.X)
                    negm = small.tile([4, 1], F32, name="negm", tag="sm1")
                    nc.vector.tensor_scalar_mul(out=negm, in0=msum, scalar1=-1.0 / N)
                    dxf = small.tile([4, N], F32, name="dxf", tag="sm")
                    nc.vector.tensor_scalar_add(out=dxf, in0=dx, scalar1=negm)
                    nc.sync.dma_start(
                        out=out_dx[2 * p].transpose([1, 0]), in_=dxf[0:2, :]
                    )
                    nc.sync.dma_start(
                        out=out_dx[2 * p + 1].transpose([1, 0]), in_=dxf[2:4, :]
                    )

                stages = [stage1, stage2, stage3, stage4]
                order = [int(c) for c in os.environ.get("BACKFLOW_ORDER", "0132")]
                for t in range(PAIRS + len(stages) - 1):
                    for si in order:
                        p = t - si
                        if 0 <= p < PAIRS:
                            stages[si](p)

    patched = _legalize_sync(nc.to_json_bytes())
    nc.to_json_bytes = lambda: patched
    return nc


def _prep_weights(inputs):
    f8 = np.float64
    g = {k: np.asarray(v, dtype=f8) for k, v in inputs.items()}
    inv = 1.0 / (N - 1)

    wtop0 = g["v2e_w1"][0][:H]
    wbot0 = g["v2e_w1"][0][H:]
    wtop1 = g["v2e_w1"][1][:H]
    wbot1 = g["v2e_w1"][1][H:]

    w_c = g["edge_w2"] @ wbot0
    b_p2 = g["edge_b2"] @ wbot0 + g["v2e_b1"][0]
    w_a0 = g["v2e_w2"][0] @ g["e2v_w1"][0]
    b_p3 = g["v2e_b2"][0] @ g["e2v_w1"][0] + g["e2v_b1"][0]
    w_b1 = g["v2e_w2"][0] @ wbot1
    b_p4 = g["v2e_b2"][0] @ wbot1 + g["v2e_b1"][1]
    w_a1 = g["v2e_w2"][1] @ g["e2v_w1"][1]
    b_p5 = g["v2e_b2"][1] @ g["e2v_w1"][1] + g["e2v_b1"][1]
    w_g0 = g["e2v_w2"][0] * inv
    w_g1 = g["e2v_w2"][1] * inv
    sp = float(np.log1p(np.exp(g["scale"][0])))
    w_h2 = g["head_w2"] * sp
    b_h2 = g["head_b2"] * sp

    def bd(w):  # [64,64] -> [128,128] block-diag
        o = np.zeros((128, 128), f8)
        o[:64, :64] = w
        o[64:, 64:] = w
        return o

    def dup(b):  # [64] -> [128]
        return np.concatenate([b, b])

    ws = {}
    wn1 = np.zeros((6, 128), f8)
    wn1[0:3, 0:64] = g["node_w1"]
    wn1[3:6, 64:128] = g["node_w1"]
    ws["w_node1"] = wn1
    ws["w_node2"] = bd(g["node_w2"])
    e1 = g["edge_w1"]
    wein = np.zeros((8, 128), f8)
    wein[0, 0:64] = e1[0]     # dr0 walker a
    wein[1, 0:64] = e1[1]     # dr1 walker a
    wein[2, 64:128] = e1[0]   # dr0 walker b
    wein[3, 64:128] = e1[1]   # dr1 walker b
    wein[4, 0:64] = e1[3]     # r2 walker a
    wein[5, 64:128] = e1[3]   # r2 walker b
    wein[6, 0:64] = e1[2]     # rr walker a
    wein[7, 64:128] = e1[2]   # rr walker b
    ws["w_ein"] = wein
    ws["w_c"] = bd(w_c)
    ws["w_a0"] = bd(w_a0)
    ws["w_b1"] = bd(w_b1)
    ws["w_a1"] = bd(w_a1)
    ws["w_g0"] = bd(w_g0)
    ws["w_g1"] = bd(w_g1)
    ws["w_top0"] = bd(wtop0)
    ws["w_top1"] = bd(wtop1)
    ws["w_h1"] = bd(g["head_w1"])
    wh2 = np.zeros((128, 4), f8)
    wh2[0:64, 0:2] = w_h2
    wh2[64:128, 2:4] = w_h2
    ws["w_h2"] = wh2
    ws["eye64"] = np.eye(64, dtype=f8)

    bias = np.zeros((128, 12), f8)
    bias[:, 0] = dup(g["node_b1"])
    bias[:, 1] = dup(g["node_b2"])
    bias[:, 2] = dup(g["edge_b1"])
    bias[:, 3] = dup(b_p2)
    bias[:, 4] = dup(b_p3)
    bias[:, 5] = dup(b_p4)
    bias[:, 6] = dup(b_p5)
    bias[:, 7] = dup(g["e2v_b2"][0])
    bias[:, 8] = dup(g["e2v_b2"][1])
    bias[:, 9] = dup(g["head_b1"])
    bias[0:4, 10] = [b_h2[0], b_h2[1], b_h2[0], b_h2[1]]
    ws["biases"] = bias
    out = {k: np.ascontiguousarray(v, dtype=np.float32) for k, v in ws.items()}

    def pack_bf16(a):  # [128,128] f32 -> [128,64] f32 holding 2 bf16 each
        import ml_dtypes
        u16 = np.ascontiguousarray(
            np.asarray(a, np.float32).astype(ml_dtypes.bfloat16)
        ).view(np.uint16)
        u32 = u16[:, 0::2].astype(np.uint32) | (
            u16[:, 1::2].astype(np.uint32) << 16
        )
        return np.ascontiguousarray(u32).view(np.float32)

    for k in ("w_c", "w_a0", "w_b1", "w_a1"):
        out[k + "_bf"] = pack_bf16(out[k])
    return out


def _pack_consts(ws, xt, st_):
    """Assemble the [128, 2608] wpack array (see _build column map)."""
    wp = np.zeros((128, 2608), np.float32)

    def put(col, arr, rows=128):
        a = np.asarray(arr, np.float32)
        wp[: a.shape[0], col : col + a.shape[1]] = a

    put(0, ws["w_node1"])
    put(128, ws["w_node2"])
    put(256, ws["w_ein"])
    put(384, ws["w_c"])
    put(512, ws["w_a0"])
    put(640, ws["w_b1"])
    put(768, ws["w_a1"])
    put(896, ws["w_g0"])
    put(1024, ws["w_g1"])
    put(1152, ws["w_top0"])
    put(1280, ws["w_top1"])
    put(1408, ws["w_h1"])
    put(1536, ws["w_h2"])
    put(1540, ws["eye64"])
    put(1604, ws["biases"])
    BCl, Nl = xt.shape[0], xt.shape[2]
    put(1616, xt.reshape(BCl, 2, 8, 8).transpose(0, 2, 1, 3).reshape(128, 16))
    put(1632, np.repeat(xt.reshape(BCl, 1, 2 * Nl), 8, axis=1).reshape(128, 2 * Nl))
    put(1760, xt.reshape(128, 16))
    put(1776, np.repeat(xt.reshape(BCl * 2, 1, Nl), 4, axis=1).reshape(128, Nl))
    nin = np.concatenate([xt, st_], axis=1).reshape(PAIRS, 6, Nl)
    put(1840, nin.transpose(1, 0, 2).reshape(6, 8 * Nl))
    put(2352, ws["w_c_bf"])
    put(2416, ws["w_a0_bf"])
    put(2480, ws["w_b1_bf"])
    put(2544, ws["w_a1_bf"])
    return wp


def kernel(**inputs) -> np.ndarray:
    x = np.asarray(inputs["x"], dtype=np.float32)       # [B, N, D]
    spin = np.asarray(inputs["spin"], dtype=np.float32) # [B, N, 1]
    ws = _prep_weights(inputs)

    if "nc" not in _BUILT:
        _BUILT["nc"] = _build()
    nc = _BUILT["nc"]

    in_maps = []
    for c in range(NCORES):
        xc = x[c * BC : (c + 1) * BC]                     # [16, N, 2]
        sc = spin[c * BC : (c + 1) * BC]                  # [16, N, 1]
        xt = np.ascontiguousarray(xc.transpose(0, 2, 1))  # [16, 2, N]
        st = np.ascontiguousarray(sc.transpose(0, 2, 1))  # [16, 1, N]
        in_maps.append({"wpack": _pack_consts(ws, xt, st)})

    res = run_bass_kernel_spmd(
        nc,
        in_maps,
        core_ids=list(range(NCORES)),
        trace=os.environ.get("BACKFLOW_TRACE", "0") == "1",
    )
    kernel.last_results = res
    out = np.concatenate([r["out_dx"] for r in res.results], axis=0)
    return out.astype(np.float32)



# revision 16
# speedup vs baseline: 1.2914x; 1.0659x over previous
"""BackflowNet GNN message-passing kernel for 8x Trainium2 NeuronCores.

Data-parallel over the walker axis B=128 -> 16 walkers per core, processed as
8 "pairs" (2 walkers block-diag-packed into the 128-partition dim).

Math restructuring (exact, host-side weight folding only):
  he0 = gelu(ein @ ew1 + eb1) @ ew2 + eb2           -> keep g_e = gelu(...)
  layer l: z = hv[:,i]@Wtop + he@Wbot + b1          (he = prev g @ w2 + b2 folded)
           g = gelu(z); he' = g @ w2 + b2
           m = gelu(he' @ e2v_w1 + e2v_b1)          (per-edge)
           hv += (sum_{i!=j} m_i) @ e2v_w2/(N-1) + e2v_b2   (sum moved before w2)
  head: dx = tanh(hv@hw1+hb1) @ (hw2*sp) + hb2*sp;  out = dx - mean_j dx
Per-edge tensors are feature-major [feat(part), e] with e = j*64 + i
(i = source = innermost so the aggregation is an innermost DVE reduce).

Matmuls run in float32r (FP22 in the PE array, 4x faster than fp32); every
tensor the PE consumes is declared float32r so the BIR verifier sees rounded
producers.
"""

import math
import os

import numpy as np

import concourse.bass as bass
import concourse.mybir as mybir
import concourse.tile as tile
import concourse.bass_utils as _bu
from concourse.bass_utils import run_bass_kernel_spmd

if os.environ.get("BACKFLOW_LDW_OPT", "0") == "1" and not getattr(_bu, "_ldw_patched", False):
    _bu._ldw_patched = True
    _orig_run = _bu.run_command

    def _run(cmd, cwd=None):
        if cmd and "walrus_driver" in cmd[0]:
            cmd = [c if c != "--enable-ldw-opt=false" else "--enable-ldw-opt=true"
                   for c in cmd]
        return _orig_run(cmd, cwd=cwd)

    _bu.run_command = _run

NCORES = 8
B, N, D = 128, 64, 2
H = 64
M = 64
BC = B // NCORES          # walkers per core
PAIRS = BC // 2           # walker pairs per core
E = N * N                 # edges (incl. diagonal) per walker
SUB = 512                 # matmul moving free dim
# gelu/psum blocks: uneven (3 ACT ops per pass instead of 4 cuts the
# per-op (N+352)/1.2 ns ACT overhead)
_B = int(os.environ.get("BACKFLOW_BLK", "1536"))
BLOCKS = ([(0, 1536), (1536, 1536), (3072, 1024)] if _B == 1536
          else [(0, 1024), (1024, 1024), (2048, 1024), (3072, 1024)])
PBLK = _B                 # psum tile width
F32 = mybir.dt.float32
F32R = mybir.dt.float32r
BF16 = mybir.dt.bfloat16
AF = mybir.ActivationFunctionType
AX = mybir.AxisListType

HU_DVE = os.environ.get("BACKFLOW_HU_DVE", "1") == "1"
# bf16 edge-feature tensors: gelu outputs quantized to bf16 (ACT cost is
# dtype-independent; DVE tensor_tensor gets the 2x_1p mode, matmul rhs
# bf16 is 1 cyc/row). The i-aggregation becomes 2 bf16 fold-adds (2x)
# + a short 1x reduce instead of one full-width 1x reduce.
EDGE_BF16 = os.environ.get("BACKFLOW_BF16", "1") == "1"
EDT = BF16 if EDGE_BF16 else F32R
# PE HAM warmup: ~8 back-to-back matmuls (~5us cold) fill the 4096-cycle
# activity window so the PE clock ungates to 2.4 GHz before real work.
NWARM = int(os.environ.get("BACKFLOW_WARM", "8"))

_BUILT = {}


def _legalize_sync(bir_bytes):
    """Walrus on this toolchain encodes at most one semaphore wait per
    engine instruction (none on DMA queue entries). Tile attaches as many
    waits as deps require, so spill the surplus into standalone
    EventSemaphore instructions on the same engine, placed just before."""
    import json as _json

    d = _json.loads(bir_bytes)
    n = [0]

    def fix_block(bb):
        insts = bb.get("instructions")
        if not insts:
            return
        out = []
        for ins in insts:
            si = ins.get("sync_info")
            waits = (si or {}).get("on_wait") or []
            opc = ins.get("opcode", "")
            if opc == "EventSemaphore":
                allowed = 1
            elif opc.startswith("DMA") or ins.get("queue"):
                allowed = 0
            else:
                allowed = 1
            if len(waits) > allowed:
                keep, spill = waits[:allowed], waits[allowed:]
                for w in spill:
                    n[0] += 1
                    out.append({
                        "debug": ins.get("debug", 0),
                        "engine": ins["engine"],
                        "ins": [],
                        "outs": [],
                        "name": f"evw-{n[0]}",
                        "opcode": "EventSemaphore",
                        "sync_info": {"on_update": [], "on_wait": [w]},
                    })
                si["on_wait"] = keep
            out.append(ins)
        bb["instructions"] = out

    def walk(obj):
        if isinstance(obj, dict):
            if "instructions" in obj:
                fix_block(obj)
            else:
                for v in obj.values():
                    walk(v)
        elif isinstance(obj, list):
            for v in obj:
                walk(v)

    walk(d)
    return _json.dumps(d).encode()


def _build():
    nc = bass.Bass(
        "TRN2", target_bir_lowering=False, debug=False, enable_asserts=False
    )

    # All constants + per-core data in one packed dram tensor (one DMA).
    # Column map (fp32 elements; rows = partitions):
    #  0:128    wn1 [6,128]       128:256  wn2        256:384  wein [8,128]
    #  384:512  wc                512:640  wa0        640:768  wb1
    #  768:896  wa1               896:1024 wg0        1024:1152 wg1
    #  1152:1280 wt0              1280:1408 wt1       1408:1536 wh1
    #  1536:1540 wh2 [128,4]      1540:1604 eye [64,64]
    #  1604:1616 biases [128,12]  1616:1632 xp_lhs    1632:1760 xp_rhs
    #  1760:1776 xq_lhs           1776:1840 xq_rhs    1840:2352 nin [6,512]
    WCOLS = 2352
    wpack = nc.dram_tensor("wpack", [128, WCOLS], F32R, kind="ExternalInput").ap()
    # bf16-packed wc/wa0/wb1/wa1 ride in their own fp32 tensor: an f32r
    # dram tensor gets fp22-rounded on load, which destroys the low bf16
    # of each packed pair.
    wbpack = nc.dram_tensor("wbpack", [128, 256], F32, kind="ExternalInput").ap()
    out_dx = nc.dram_tensor("out_dx", [BC, N, D], F32, kind="ExternalOutput").ap()

    with tile.TileContext(nc) as tc:
        with (
            tc.tile_pool(name="consts", bufs=1) as consts,
            tc.tile_pool(name="feat", bufs=1) as feat,
            tc.tile_pool(name="big", bufs=7) as big,
            tc.tile_pool(name="gblk", bufs=8) as gblk,
            tc.tile_pool(name="fold", bufs=4) as fold,
            tc.tile_pool(name="small", bufs=24) as small,
            tc.tile_pool(name="swide", bufs=4) as swide,
            tc.tile_pool(name="pbig", bufs=(2 if _B == 1536 else 3),
                         space="PSUM") as pbig,
            tc.tile_pool(name="psmall", bufs=2, space="PSUM") as psmall,
        ):
            wp = consts.tile([128, WCOLS], F32R, name="wp", tag="wp")
            nc.sync.dma_start(out=wp, in_=wpack)
            wn1 = wp[0:6, 0:128]
            wn2 = wp[:, 128:256]
            wein = wp[0:8, 256:384]
            wc = wp[:, 384:512]
            wa0 = wp[:, 512:640]
            wb1 = wp[:, 640:768]
            wa1 = wp[:, 768:896]
            wg0 = wp[:, 896:1024]
            wg1 = wp[:, 1024:1152]
            wt0 = wp[:, 1152:1280]
            wt1 = wp[:, 1280:1408]
            wh1 = wp[:, 1408:1536]
            wh2 = wp[:, 1536:1540]
            eye_t = wp[0:64, 1540:1604]
            wbp = consts.tile([128, 256], F32, name="wbp", tag="wbp")
            nc.sync.dma_start(out=wbp, in_=wbpack)
            wc_bf = wbp[:, 0:64].bitcast(BF16)
            wa0_bf = wbp[:, 64:128].bitcast(BF16)
            wb1_bf = wbp[:, 128:192].bitcast(BF16)
            wa1_bf = wbp[:, 192:256].bitcast(BF16)
            if EDGE_BF16:
                wc_e, wa0_e, wb1_e, wa1_e = wc_bf, wa0_bf, wb1_bf, wa1_bf
            else:
                wc_e, wa0_e, wb1_e, wa1_e = wc, wa0, wb1, wa1
            bia = wp[:, 1604:1616].bitcast(F32)
            xpl = wp[:, 1616:1632].bitcast(F32)
            xpr = wp[:, 1632:1760].bitcast(F32)
            xql = wp[:, 1760:1776].bitcast(F32)
            xqr = wp[:, 1776:1840].bitcast(F32)
            nin = wp[0:6, 1840:2352]
            eps_t = consts.tile([128, 1], F32, name="eps_t", tag="eps_t")
            nc.vector.memset(eps_t, 1e-12)

            if NWARM:
                warm_w = consts.tile([128, 128], F32R, name="warm_w", tag="warm")
                nc.vector.memset(warm_w.bitcast(F32), 0.0)
                wmp = psmall.tile([128, 512], F32, name="wmp", tag="ps_s")
                warm_rhs = warm_w.unsqueeze(1).broadcast_to([128, 4, 128])
                for k in range(NWARM):
                    nc.tensor.matmul(
                        wmp, warm_w, warm_rhs,
                        start=(k == 0), stop=(k == NWARM - 1),
                    )

            # identity-broadcast rhs: rhs[k,(j,i)] = eye[k,i], j in 8-block
            eye_bc = eye_t.unsqueeze(1).broadcast_to([64, SUB // N, 64])


            import contextlib
            rep = int(os.environ.get("BACKFLOW_REPEAT", "1"))
            loop_cm = tc.For_i(0, rep, 1) if rep > 1 else contextlib.nullcontext()
            with loop_cm:
                # -------- phase 0: dr / r2 / rr in packed layouts ---------------
                # (w,d,jq2)-packed dr for the ein rows (contiguous per pair)
                dre_pk = feat.tile([128, 1024], F32R, name="dre_pk", tag="drepk")
                nc.vector.tensor_sub(
                    out=dre_pk.rearrange("p (j i) -> p j i", i=N),
                    in0=xql.unsqueeze(2).broadcast_to([128, 16, N]),
                    in1=xqr.unsqueeze(1).broadcast_to([128, 16, N]),
                )
                # (w,jq)-packed dr for r2/rr
                dr_pk = feat.tile([128, 1024], F32, name="dr_pk", tag="drpk")
                nc.vector.tensor_sub(
                    out=dr_pk.rearrange("p (d j i) -> p d j i", d=2, i=N),
                    in0=xpl.rearrange("p (d j) -> p d j", d=2)
                        .unsqueeze(3).broadcast_to([128, 2, 8, N]),
                    in1=xpr.rearrange("p (d i) -> p d i", d=2)
                        .unsqueeze(2).broadcast_to([128, 2, 8, N]),
                )
                sq_pk = feat.tile([128, 1024], F32, name="sq_pk", tag="sqpk")
                nc.vector.tensor_mul(out=sq_pk, in0=dr_pk, in1=dr_pk)
                sqv = sq_pk.rearrange("p (d f) -> p d f", d=2)
                r2_pk = feat.tile([128, 512], F32R, name="r2_pk", tag="r2pk")
                nc.vector.tensor_add(out=r2_pk, in0=sqv[:, 0, :], in1=sqv[:, 1, :])
                rr_pk = feat.tile([128, 512], F32R, name="rr_pk", tag="rrpk")
                sqrt_inst = nc.scalar.activation(
                    out=rr_pk, in_=r2_pk, func=AF.Sqrt, bias=eps_t, scale=1.0
                )

                # -------- batched node MLP (all 8 pairs) ------------------------
                zn = psmall.tile([128, 8 * N], F32, name="zn", tag="ps_s")
                zn_inst = nc.tensor.matmul(zn, wn1, nin)
                from concourse.tile_rust import add_dep_helper
                add_dep_helper(zn_inst.ins, sqrt_inst.ins,
                               reason="keep sqrt first in ACT stream (table set)")
                gn = swide.tile([128, 8 * N], F32R, name="gn", tag="sw")
                nc.scalar.activation(
                    out=gn, in_=zn, func=AF.Gelu, bias=bia[:, 0:1], scale=1.0
                )
                hv0p = psmall.tile([128, 8 * N], F32, name="hv0p", tag="ps_s")
                nc.tensor.matmul(hv0p, wn2, gn)
                hv0a = swide.tile([128, 8 * N], F32R, name="hv0a", tag="sw")
                nc.vector.tensor_scalar_add(out=hv0a, in0=hv0p, scalar1=bia[:, 1:2])

                # -------- software-pipelined per-pair stages --------------------
                st = [dict() for _ in range(PAIRS)]

                def edge_pass_blk(bl, dst, srcs, gelu_bias, hu_m=None):
                    lo0, blen = BLOCKS[bl]
                    ps = pbig.tile([128, PBLK], F32, name="ps", tag="ps_b")
                    for s in range(blen // SUB):
                        lo = lo0 + s * SUB
                        out_sl = ps[:, s * SUB : (s + 1) * SUB]
                        for k, (lhsT, rhs_fn) in enumerate(srcs):
                            nc.tensor.matmul(
                                out_sl,
                                lhsT,
                                rhs_fn(lo),
                                start=(k == 0),
                                stop=(k == len(srcs) - 1),
                            )
                    if hu_m is not None:
                        psv = ps[:, 0:blen].rearrange("p (j i) -> p j i", i=N)
                        nc.vector.tensor_add(
                            out=psv, in0=psv,
                            in1=hu_m.unsqueeze(1).broadcast_to(
                                [128, blen // N, N]
                            ),
                        )
                    nc.scalar.activation(
                        out=dst[:, 0:blen], in_=ps[:, 0:blen], func=AF.Gelu,
                        bias=gelu_bias, scale=1.0,
                    )

                def agg_layer(src_g, wagg, bias_col, acc_name):
                    acc = small.tile([128, N], F32, name=acc_name, tag="sm")
                    accd = small.tile([128, N], F32R, name=acc_name + "d", tag="sm")
                    mjb = PBLK // N
                    for bl, (lo0, blen) in enumerate(BLOCKS):
                        gb = gblk.tile([128, PBLK], EDT, name="gb", tag="gb")
                        edge_pass_blk(
                            bl, gb,
                            [(wagg, lambda lo: src_g[:, lo : lo + SUB])],
                            bias_col,
                        )
                        jlo = lo0 // N
                        jb = blen // N
                        gv = gb[:, 0:blen].rearrange("p (j i) -> p j i", i=N)
                        if EDGE_BF16:
                            f1 = fold.tile([128, mjb, 32], BF16, name="f1",
                                           tag="f1")
                            nc.vector.tensor_add(
                                out=f1[:, 0:jb, :], in0=gv[:, :, 0:32],
                                in1=gv[:, :, 32:64],
                            )
                            f2 = fold.tile([128, mjb, 16], BF16, name="f2",
                                           tag="f2")
                            nc.vector.tensor_add(
                                out=f2[:, 0:jb, :], in0=f1[:, 0:jb, 0:16],
                                in1=f1[:, 0:jb, 16:32],
                            )
                            nc.vector.reduce_sum(
                                out=acc[:, jlo : jlo + jb], in_=f2[:, 0:jb, :],
                                axis=AX.X,
                            )
                        else:
                            nc.vector.reduce_sum(
                                out=acc[:, jlo : jlo + jb], in_=gv, axis=AX.X,
                            )
                        diag = gb[:, jlo : jlo + (jb - 1) * (N + 1) + 1 : N + 1]
                        nc.vector.tensor_sub(
                            out=accd[:, jlo : jlo + jb],
                            in0=acc[:, jlo : jlo + jb],
                            in1=diag,
                        )
                    return accd

                def stage1(p):
                    s = st[p]
                    hv0 = hv0a[:, p * N : (p + 1) * N]
                    s["hv0"] = hv0
                    if HU_DVE:
                        # hu_m[m, i] = (Wtop0_bd.T @ hv0)[m, i] for DVE bcast-add
                        hu0p = psmall.tile([128, 64], F32, name="hu0p", tag="ps_s")
                        nc.tensor.matmul(hu0p, wt0, hv0)
                        hu0 = small.tile([128, 64], F32, name="hu0", tag="sm")
                        nc.vector.tensor_copy(out=hu0, in_=hu0p)
                    else:
                        hu0p = psmall.tile([64, 128], F32, name="hu0p", tag="ps_s")
                        nc.tensor.matmul(hu0p, hv0, wt0)
                        hu0 = small.tile([64, 128], F32R, name="hu0", tag="sm")
                        nc.vector.tensor_copy(out=hu0, in_=hu0p)
                    if EDGE_BF16:
                        ein = big.tile([8, E], F32R, name="ein", tag="ein",
                                       bufs=3)
                    else:
                        ein = big.tile([8, E], F32R, name="ein", tag="stream")
                    nc.gpsimd.dma_start(
                        out=ein[0:4, :], in_=dre_pk[16 * p : 16 * p + 16, :]
                    )
                    nc.gpsimd.dma_start(
                        out=ein[4:6, :], in_=r2_pk[16 * p : 16 * p + 16, :]
                    )
                    nc.gpsimd.dma_start(
                        out=ein[6:8, :], in_=rr_pk[16 * p : 16 * p + 16, :]
                    )
                    g1 = big.tile([128, E], EDT, name="g1", tag="stream")
                    s["g1"] = g1
                    for bl, (lo0, blen) in enumerate(BLOCKS):
                        geb = gblk.tile([128, PBLK], EDT, name="geb", tag="gb")
                        edge_pass_blk(
                            bl, geb,
                            [(wein, lambda lo: ein[:, lo : lo + SUB])],
                            bia[:, 2:3],
                        )
                        if HU_DVE:
                            edge_pass_blk(
                                bl, g1[:, lo0 : lo0 + blen],
                                [(wc_e, lambda lo: geb[:, lo - lo0 : lo - lo0 + SUB])],
                                bia[:, 3:4], hu_m=hu0,
                            )
                        else:
                            edge_pass_blk(
                                bl, g1[:, lo0 : lo0 + blen],
                                [
                                    (wc_e, lambda lo: geb[:, lo - lo0 : lo - lo0 + SUB]),
                                    (hu0, lambda lo: eye_bc),
                                ],
                                bia[:, 3:4],
                            )

                def stage2(p):
                    s = st[p]
                    accd0 = agg_layer(s["g1"], wa0_e, bia[:, 4:5], "acc0")
                    u0p = psmall.tile([128, N], F32, name="u0p", tag="ps_s")
                    nc.tensor.matmul(u0p, wg0, accd0)
                    hv1 = small.tile([128, N], F32R, name="hv1", tag="sm")
                    nc.vector.scalar_tensor_tensor(
                        out=hv1, in0=u0p, scalar=bia[:, 7:8], in1=s["hv0"],
                        op0=mybir.AluOpType.add, op1=mybir.AluOpType.add,
                    )
                    s["hv1"] = hv1
                    if HU_DVE:
                        hu1p = psmall.tile([128, 64], F32, name="hu1p", tag="ps_s")
                        nc.tensor.matmul(hu1p, wt1, hv1)
                        hu1 = small.tile([128, 64], F32, name="hu1", tag="sm")
                        nc.vector.tensor_copy(out=hu1, in_=hu1p)
                    else:
                        hu1p = psmall.tile([64, 128], F32, name="hu1p", tag="ps_s")
                        nc.tensor.matmul(hu1p, hv1, wt1)
                        hu1 = small.tile([64, 128], F32R, name="hu1", tag="sm")
                        nc.vector.tensor_copy(out=hu1, in_=hu1p)
                    s["hu1"] = hu1

                def stage3(p):
                    s = st[p]
                    g1 = s["g1"]
                    hu1 = s["hu1"]
                    g3 = big.tile([128, E], EDT, name="g3", tag="stream")
                    s["g3"] = g3
                    for bl, (lo0, blen) in enumerate(BLOCKS):
                        if HU_DVE:
                            edge_pass_blk(
                                bl, g3[:, lo0 : lo0 + blen],
                                [(wb1_e, lambda lo: g1[:, lo : lo + SUB])],
                                bia[:, 5:6], hu_m=hu1,
                            )
                        else:
                            edge_pass_blk(
                                bl, g3[:, lo0 : lo0 + blen],
                                [
                                    (wb1_e, lambda lo: g1[:, lo : lo + SUB]),
                                    (hu1, lambda lo: eye_bc),
                                ],
                                bia[:, 5:6],
                            )

                def stage4(p):
                    s = st[p]
                    accd1 = agg_layer(s["g3"], wa1_e, bia[:, 6:7], "acc1")
                    u1p = psmall.tile([128, N], F32, name="u1p", tag="ps_s")
                    nc.tensor.matmul(u1p, wg1, accd1)
                    hv2 = small.tile([128, N], F32R, name="hv2", tag="sm")
                    nc.vector.scalar_tensor_tensor(
                        out=hv2, in0=u1p, scalar=bia[:, 8:9], in1=s["hv1"],
                        op0=mybir.AluOpType.add, op1=mybir.AluOpType.add,
                    )
                    thp = psmall.tile([128, N], F32, name="thp", tag="ps_s")
                    nc.tensor.matmul(thp, wh1, hv2)
                    th = small.tile([128, N], F32R, name="th", tag="sm")
                    nc.scalar.activation(
                        out=th, in_=thp, func=AF.Tanh, bias=bia[:, 9:10], scale=1.0
                    )
                    dxp = psmall.tile([4, N], F32, name="dxp", tag="ps_s")
                    nc.tensor.matmul(dxp, wh2, th)
                    dx = small.tile([4, N], F32, name="dx", tag="sm")
                    nc.vector.tensor_scalar_add(out=dx, in0=dxp, scalar1=bia[0:4, 10:11])
                    msum = small.tile([4, 1], F32, name="msum", tag="sm1")
                    nc.vector.reduce_sum(out=msum, in_=dx, axis=AX.X)
                    negm = small.tile([4, 1], F32, name="negm", tag="sm1")
                    nc.vector.tensor_scalar_mul(out=negm, in0=msum, scalar1=-1.0 / N)
                    dxf = small.tile([4, N], F32, name="dxf", tag="sm")
                    nc.vector.tensor_scalar_add(out=dxf, in0=dx, scalar1=negm)
                    nc.sync.dma_start(
                        out=out_dx[2 * p].transpose([1, 0]), in_=dxf[0:2, :]
                    )
                    nc.sync.dma_start(
                        out=out_dx[2 * p + 1].transpose([1, 0]), in_=dxf[2:4, :]
                    )

                stages = [stage1, stage2, stage3, stage4]
                order = [int(c) for c in os.environ.get("BACKFLOW_ORDER", "0132")]
                for t in range(PAIRS + len(stages) - 1):
                    for si in order:
                        p = t - si
                        if 0 <= p < PAIRS:
                            stages[si](p)

    patched = _legalize_sync(nc.to_json_bytes())
    nc.to_json_bytes = lambda: patched
    return nc


def _prep_weights(inputs):
    f8 = np.float64
    g = {k: np.asarray(v, dtype=f8) for k, v in inputs.items()}
    inv = 1.0 / (N - 1)

    wtop0 = g["v2e_w1"][0][:H]
    wbot0 = g["v2e_w1"][0][H:]
    wtop1 = g["v2e_w1"][1][:H]
    wbot1 = g["v2e_w1"][1][H:]

    w_c = g["edge_w2"] @ wbot0
    b_p2 = g["edge_b2"] @ wbot0 + g["v2e_b1"][0]
    w_a0 = g["v2e_w2"][0] @ g["e2v_w1"][0]
    b_p3 = g["v2e_b2"][0] @ g["e2v_w1"][0] + g["e2v_b1"][0]
    w_b1 = g["v2e_w2"][0] @ wbot1
    b_p4 = g["v2e_b2"][0] @ wbot1 + g["v2e_b1"][1]
    w_a1 = g["v2e_w2"][1] @ g["e2v_w1"][1]
    b_p5 = g["v2e_b2"][1] @ g["e2v_w1"][1] + g["e2v_b1"][1]
    w_g0 = g["e2v_w2"][0] * inv
    w_g1 = g["e2v_w2"][1] * inv
    sp = float(np.log1p(np.exp(g["scale"][0])))
    w_h2 = g["head_w2"] * sp
    b_h2 = g["head_b2"] * sp

    def bd(w):  # [64,64] -> [128,128] block-diag
        o = np.zeros((128, 128), f8)
        o[:64, :64] = w
        o[64:, 64:] = w
        return o

    def dup(b):  # [64] -> [128]
        return np.concatenate([b, b])

    ws = {}
    wn1 = np.zeros((6, 128), f8)
    wn1[0:3, 0:64] = g["node_w1"]
    wn1[3:6, 64:128] = g["node_w1"]
    ws["w_node1"] = wn1
    ws["w_node2"] = bd(g["node_w2"])
    e1 = g["edge_w1"]
    wein = np.zeros((8, 128), f8)
    wein[0, 0:64] = e1[0]     # dr0 walker a
    wein[1, 0:64] = e1[1]     # dr1 walker a
    wein[2, 64:128] = e1[0]   # dr0 walker b
    wein[3, 64:128] = e1[1]   # dr1 walker b
    wein[4, 0:64] = e1[3]     # r2 walker a
    wein[5, 64:128] = e1[3]   # r2 walker b
    wein[6, 0:64] = e1[2]     # rr walker a
    wein[7, 64:128] = e1[2]   # rr walker b
    ws["w_ein"] = wein
    ws["w_c"] = bd(w_c)
    ws["w_a0"] = bd(w_a0)
    ws["w_b1"] = bd(w_b1)
    ws["w_a1"] = bd(w_a1)
    ws["w_g0"] = bd(w_g0)
    ws["w_g1"] = bd(w_g1)
    ws["w_top0"] = bd(wtop0)
    ws["w_top1"] = bd(wtop1)
    ws["w_h1"] = bd(g["head_w1"])
    wh2 = np.zeros((128, 4), f8)
    wh2[0:64, 0:2] = w_h2
    wh2[64:128, 2:4] = w_h2
    ws["w_h2"] = wh2
    ws["eye64"] = np.eye(64, dtype=f8)

    bias = np.zeros((128, 12), f8)
    bias[:, 0] = dup(g["node_b1"])
    bias[:, 1] = dup(g["node_b2"])
    bias[:, 2] = dup(g["edge_b1"])
    bias[:, 3] = dup(b_p2)
    bias[:, 4] = dup(b_p3)
    bias[:, 5] = dup(b_p4)
    bias[:, 6] = dup(b_p5)
    bias[:, 7] = dup(g["e2v_b2"][0])
    bias[:, 8] = dup(g["e2v_b2"][1])
    bias[:, 9] = dup(g["head_b1"])
    bias[0:4, 10] = [b_h2[0], b_h2[1], b_h2[0], b_h2[1]]
    ws["biases"] = bias
    out = {k: np.ascontiguousarray(v, dtype=np.float32) for k, v in ws.items()}

    def pack_bf16(a):  # [128,128] f32 -> [128,64] f32 holding 2 bf16 each
        import ml_dtypes
        u16 = np.ascontiguousarray(
            np.asarray(a, np.float32).astype(ml_dtypes.bfloat16)
        ).view(np.uint16)
        u32 = u16[:, 0::2].astype(np.uint32) | (
            u16[:, 1::2].astype(np.uint32) << 16
        )
        return np.ascontiguousarray(u32).view(np.float32)

    for k in ("w_c", "w_a0", "w_b1", "w_a1"):
        out[k + "_bf"] = pack_bf16(out[k])
    return out


def _pack_consts(ws, xt, st_):
    """Assemble the [128, 2352] wpack array (see _build column map)."""
    wp = np.zeros((128, 2352), np.float32)

    def put(col, arr, rows=128):
        a = np.asarray(arr, np.float32)
        wp[: a.shape[0], col : col + a.shape[1]] = a

    put(0, ws["w_node1"])
    put(128, ws["w_node2"])
    put(256, ws["w_ein"])
    put(384, ws["w_c"])
    put(512, ws["w_a0"])
    put(640, ws["w_b1"])
    put(768, ws["w_a1"])
    put(896, ws["w_g0"])
    put(1024, ws["w_g1"])
    put(1152, ws["w_top0"])
    put(1280, ws["w_top1"])
    put(1408, ws["w_h1"])
    put(1536, ws["w_h2"])
    put(1540, ws["eye64"])
    put(1604, ws["biases"])
    BCl, Nl = xt.shape[0], xt.shape[2]
    put(1616, xt.reshape(BCl, 2, 8, 8).transpose(0, 2, 1, 3).reshape(128, 16))
    put(1632, np.repeat(xt.reshape(BCl, 1, 2 * Nl), 8, axis=1).reshape(128, 2 * Nl))
    put(1760, xt.reshape(128, 16))
    put(1776, np.repeat(xt.reshape(BCl * 2, 1, Nl), 4, axis=1).reshape(128, Nl))
    nin = np.concatenate([xt, st_], axis=1).reshape(PAIRS, 6, Nl)
    put(1840, nin.transpose(1, 0, 2).reshape(6, 8 * Nl))
    return wp


def _pack_bf_consts(ws):
    wb = np.zeros((128, 256), np.float32)
    wb[:, 0:64] = ws["w_c_bf"]
    wb[:, 64:128] = ws["w_a0_bf"]
    wb[:, 128:192] = ws["w_b1_bf"]
    wb[:, 192:256] = ws["w_a1_bf"]
    return wb


def kernel(**inputs) -> np.ndarray:
    x = np.asarray(inputs["x"], dtype=np.float32)       # [B, N, D]
    spin = np.asarray(inputs["spin"], dtype=np.float32) # [B, N, 1]
    ws = _prep_weights(inputs)

    if "nc" not in _BUILT:
        _BUILT["nc"] = _build()
    nc = _BUILT["nc"]

    in_maps = []
    for c in range(NCORES):
        xc = x[c * BC : (c + 1) * BC]                     # [16, N, 2]
        sc = spin[c * BC : (c + 1) * BC]                  # [16, N, 1]
        xt = np.ascontiguousarray(xc.transpose(0, 2, 1))  # [16, 2, N]
        st = np.ascontiguousarray(sc.transpose(0, 2, 1))  # [16, 1, N]
        in_maps.append({"wpack": _pack_consts(ws, xt, st),
                        "wbpack": _pack_bf_consts(ws)})

    res = run_bass_kernel_spmd(
        nc,
        in_maps,
        core_ids=list(range(NCORES)),
        trace=os.environ.get("BACKFLOW_TRACE", "0") == "1",
    )
    kernel.last_results = res
    out = np.concatenate([r["out_dx"] for r in res.results], axis=0)
    return out.astype(np.float32)



# revision 19
# speedup vs baseline: 1.3051x; 1.0106x over previous
"""BackflowNet GNN message-passing kernel for 8x Trainium2 NeuronCores.

Data-parallel over the walker axis B=128 -> 16 walkers per core, processed as
8 "pairs" (2 walkers block-diag-packed into the 128-partition dim).

Math restructuring (exact, host-side weight folding only):
  he0 = gelu(ein @ ew1 + eb1) @ ew2 + eb2           -> keep g_e = gelu(...)
  layer l: z = hv[:,i]@Wtop + he@Wbot + b1          (he = prev g @ w2 + b2 folded)
           g = gelu(z); he' = g @ w2 + b2
           m = gelu(he' @ e2v_w1 + e2v_b1)          (per-edge)
           hv += (sum_{i!=j} m_i) @ e2v_w2/(N-1) + e2v_b2   (sum moved before w2)
  head: dx = tanh(hv@hw1+hb1) @ (hw2*sp) + hb2*sp;  out = dx - mean_j dx
Per-edge tensors are feature-major [feat(part), e] with e = j*64 + i
(i = source = innermost so the aggregation is an innermost DVE reduce).

Matmuls run in float32r (FP22 in the PE array, 4x faster than fp32); every
tensor the PE consumes is declared float32r so the BIR verifier sees rounded
producers.
"""

import math
import os

import numpy as np

import concourse.bass as bass
import concourse.mybir as mybir
import concourse.tile as tile
import concourse.bass_utils as _bu
from concourse.bass_utils import run_bass_kernel_spmd

if os.environ.get("BACKFLOW_LDW_OPT", "0") == "1" and not getattr(_bu, "_ldw_patched", False):
    _bu._ldw_patched = True
    _orig_run = _bu.run_command

    def _run(cmd, cwd=None):
        if cmd and "walrus_driver" in cmd[0]:
            cmd = [c if c != "--enable-ldw-opt=false" else "--enable-ldw-opt=true"
                   for c in cmd]
        return _orig_run(cmd, cwd=cwd)

    _bu.run_command = _run

NCORES = 8
B, N, D = 128, 64, 2
H = 64
M = 64
BC = B // NCORES          # walkers per core
PAIRS = BC // 2           # walker pairs per core
E = N * N                 # edges (incl. diagonal) per walker
SUB = 512                 # matmul moving free dim
# gelu/psum blocks: uneven (3 ACT ops per pass instead of 4 cuts the
# per-op (N+352)/1.2 ns ACT overhead)
_B = int(os.environ.get("BACKFLOW_BLK", "1536"))
BLOCKS = ([(0, 1536), (1536, 1536), (3072, 1024)] if _B == 1536
          else [(0, 1024), (1024, 1024), (2048, 1024), (3072, 1024)])
PBLK = _B                 # psum tile width
F32 = mybir.dt.float32
F32R = mybir.dt.float32r
BF16 = mybir.dt.bfloat16
AF = mybir.ActivationFunctionType
AX = mybir.AxisListType

HU_DVE = os.environ.get("BACKFLOW_HU_DVE", "1") == "1"
# bf16 edge-feature tensors: gelu outputs quantized to bf16 (ACT cost is
# dtype-independent; DVE tensor_tensor gets the 2x_1p mode, matmul rhs
# bf16 is 1 cyc/row). The i-aggregation becomes 2 bf16 fold-adds (2x)
# + a short 1x reduce instead of one full-width 1x reduce.
EDGE_BF16 = os.environ.get("BACKFLOW_BF16", "1") == "1"
EDT = BF16 if EDGE_BF16 else F32R
# PE HAM warmup: ~8 back-to-back matmuls (~5us cold) fill the 4096-cycle
# activity window so the PE clock ungates to 2.4 GHz before real work.
NWARM = int(os.environ.get("BACKFLOW_WARM", "8"))

_BUILT = {}


def _legalize_sync(bir_bytes):
    """Walrus on this toolchain encodes at most one semaphore wait per
    engine instruction (none on DMA queue entries). Tile attaches as many
    waits as deps require, so spill the surplus into standalone
    EventSemaphore instructions on the same engine, placed just before."""
    import json as _json

    d = _json.loads(bir_bytes)
    n = [0]

    def fix_block(bb):
        insts = bb.get("instructions")
        if not insts:
            return
        out = []
        for ins in insts:
            si = ins.get("sync_info")
            waits = (si or {}).get("on_wait") or []
            opc = ins.get("opcode", "")
            if opc == "EventSemaphore":
                allowed = 1
            elif opc.startswith("DMA") or ins.get("queue"):
                allowed = 0
            else:
                allowed = 1
            if len(waits) > allowed:
                keep, spill = waits[:allowed], waits[allowed:]
                for w in spill:
                    n[0] += 1
                    out.append({
                        "debug": ins.get("debug", 0),
                        "engine": ins["engine"],
                        "ins": [],
                        "outs": [],
                        "name": f"evw-{n[0]}",
                        "opcode": "EventSemaphore",
                        "sync_info": {"on_update": [], "on_wait": [w]},
                    })
                si["on_wait"] = keep
            out.append(ins)
        bb["instructions"] = out

    def walk(obj):
        if isinstance(obj, dict):
            if "instructions" in obj:
                fix_block(obj)
            else:
                for v in obj.values():
                    walk(v)
        elif isinstance(obj, list):
            for v in obj:
                walk(v)

    walk(d)
    return _json.dumps(d).encode()


def _build():
    nc = bass.Bass(
        "TRN2", target_bir_lowering=False, debug=False, enable_asserts=False
    )

    # All constants + per-core data in one packed dram tensor (one DMA).
    # Column map (fp32 elements; rows = partitions):
    #  0:128    wn1 [6,128]       128:256  wn2        256:384  wein [8,128]
    #  384:512  wc                512:640  wa0        640:768  wb1
    #  768:896  wa1               896:1024 wg0        1024:1152 wg1
    #  1152:1280 wt0              1280:1408 wt1       1408:1536 wh1
    #  1536:1540 wh2 [128,4]      1540:1604 eye [64,64]
    #  1604:1616 biases [128,12]  1616:1632 xp_lhs    1632:1760 xp_rhs
    #  1760:1776 xq_lhs           1776:1840 xq_rhs    1840:2352 nin [6,512]
    WCOLS = 2352
    wpack = nc.dram_tensor("wpack", [128, WCOLS], F32R, kind="ExternalInput").ap()
    # bf16-packed wc/wa0/wb1/wa1 ride in their own fp32 tensor: an f32r
    # dram tensor gets fp22-rounded on load, which destroys the low bf16
    # of each packed pair.
    wbpack = nc.dram_tensor("wbpack", [128, 256], F32, kind="ExternalInput").ap()
    out_dx = nc.dram_tensor("out_dx", [BC, N, D], F32, kind="ExternalOutput").ap()

    with tile.TileContext(nc) as tc:
        with (
            tc.tile_pool(name="consts", bufs=1) as consts,
            tc.tile_pool(name="feat", bufs=1) as feat,
            tc.tile_pool(name="big", bufs=7) as big,
            tc.tile_pool(name="gblk", bufs=8) as gblk,
            tc.tile_pool(name="fold", bufs=4) as fold,
            tc.tile_pool(name="small", bufs=24) as small,
            tc.tile_pool(name="swide", bufs=4) as swide,
            tc.tile_pool(name="pbig", bufs=(2 if _B == 1536 else 3),
                         space="PSUM") as pbig,
            tc.tile_pool(name="psmall", bufs=2, space="PSUM") as psmall,
        ):
            wp = consts.tile([128, WCOLS], F32R, name="wp", tag="wp")
            nc.sync.dma_start(out=wp, in_=wpack)
            wn1 = wp[0:6, 0:128]
            wn2 = wp[:, 128:256]
            wein = wp[0:8, 256:384]
            wc = wp[:, 384:512]
            wa0 = wp[:, 512:640]
            wb1 = wp[:, 640:768]
            wa1 = wp[:, 768:896]
            wg0 = wp[:, 896:1024]
            wg1 = wp[:, 1024:1152]
            wt0 = wp[:, 1152:1280]
            wt1 = wp[:, 1280:1408]
            wh1 = wp[:, 1408:1536]
            wh2 = wp[:, 1536:1540]
            eye_t = wp[0:64, 1540:1604]
            wbp = consts.tile([128, 256], F32, name="wbp", tag="wbp")
            nc.sync.dma_start(out=wbp, in_=wbpack)
            wc_bf = wbp[:, 0:64].bitcast(BF16)
            wa0_bf = wbp[:, 64:128].bitcast(BF16)
            wb1_bf = wbp[:, 128:192].bitcast(BF16)
            wa1_bf = wbp[:, 192:256].bitcast(BF16)
            if EDGE_BF16:
                wc_e, wa0_e, wb1_e, wa1_e = wc_bf, wa0_bf, wb1_bf, wa1_bf
            else:
                wc_e, wa0_e, wb1_e, wa1_e = wc, wa0, wb1, wa1
            bia = wp[:, 1604:1616].bitcast(F32)
            xpl = wp[:, 1616:1632].bitcast(F32)
            xpr = wp[:, 1632:1760].bitcast(F32)
            xql = wp[:, 1760:1776].bitcast(F32)
            xqr = wp[:, 1776:1840].bitcast(F32)
            nin = wp[0:6, 1840:2352]
            eps_t = consts.tile([128, 1], F32, name="eps_t", tag="eps_t")
            nc.vector.memset(eps_t, 1e-12)

            if NWARM:
                # HAM warmup: a back-to-back MM burst long enough to fill a
                # 4096-cycle activity window promotes the PE clock gate to
                # 2.4 GHz. Reading wp makes it start only after the const
                # DMA, so it overlaps phase-0/node-MLP prep and hands the
                # warm state directly into the edge-pass pipeline (the
                # steady state never leaves a window fully idle, so warm
                # sticks).
                wmp = psmall.tile([128, 512], F32, name="wmp", tag="ps_s")
                warm_rhs = wn2.unsqueeze(1).broadcast_to([128, 4, 128])
                for k in range(NWARM):
                    nc.tensor.matmul(
                        wmp, wn2, warm_rhs,
                        start=(k == 0), stop=(k == NWARM - 1),
                    )

            # identity-broadcast rhs: rhs[k,(j,i)] = eye[k,i], j in 8-block
            eye_bc = eye_t.unsqueeze(1).broadcast_to([64, SUB // N, 64])


            import contextlib
            rep = int(os.environ.get("BACKFLOW_REPEAT", "1"))
            loop_cm = tc.For_i(0, rep, 1) if rep > 1 else contextlib.nullcontext()
            with loop_cm:
                # -------- phase 0: dr / r2 / rr in packed layouts ---------------
                # (w,d,jq2)-packed dr for the ein rows (contiguous per pair)
                dre_pk = feat.tile([128, 1024], F32R, name="dre_pk", tag="drepk")
                nc.vector.tensor_sub(
                    out=dre_pk.rearrange("p (j i) -> p j i", i=N),
                    in0=xql.unsqueeze(2).broadcast_to([128, 16, N]),
                    in1=xqr.unsqueeze(1).broadcast_to([128, 16, N]),
                )
                # (w,jq)-packed dr for r2/rr
                dr_pk = feat.tile([128, 1024], F32, name="dr_pk", tag="drpk")
                nc.vector.tensor_sub(
                    out=dr_pk.rearrange("p (d j i) -> p d j i", d=2, i=N),
                    in0=xpl.rearrange("p (d j) -> p d j", d=2)
                        .unsqueeze(3).broadcast_to([128, 2, 8, N]),
                    in1=xpr.rearrange("p (d i) -> p d i", d=2)
                        .unsqueeze(2).broadcast_to([128, 2, 8, N]),
                )
                sq_pk = feat.tile([128, 1024], F32, name="sq_pk", tag="sqpk")
                nc.vector.tensor_mul(out=sq_pk, in0=dr_pk, in1=dr_pk)
                sqv = sq_pk.rearrange("p (d f) -> p d f", d=2)
                r2_pk = feat.tile([128, 512], F32R, name="r2_pk", tag="r2pk")
                nc.vector.tensor_add(out=r2_pk, in0=sqv[:, 0, :], in1=sqv[:, 1, :])
                rr_pk = feat.tile([128, 512], F32R, name="rr_pk", tag="rrpk")
                sqrt_inst = nc.scalar.activation(
                    out=rr_pk, in_=r2_pk, func=AF.Sqrt, bias=eps_t, scale=1.0
                )

                # -------- batched node MLP (all 8 pairs) ------------------------
                zn = psmall.tile([128, 8 * N], F32, name="zn", tag="ps_s")
                zn_inst = nc.tensor.matmul(zn, wn1, nin)
                from concourse.tile_rust import add_dep_helper
                add_dep_helper(zn_inst.ins, sqrt_inst.ins,
                               reason="keep sqrt first in ACT stream (table set)")
                gn = swide.tile([128, 8 * N], F32R, name="gn", tag="sw")
                nc.scalar.activation(
                    out=gn, in_=zn, func=AF.Gelu, bias=bia[:, 0:1], scale=1.0
                )
                hv0p = psmall.tile([128, 8 * N], F32, name="hv0p", tag="ps_s")
                nc.tensor.matmul(hv0p, wn2, gn)
                hv0a = swide.tile([128, 8 * N], F32R, name="hv0a", tag="sw")
                nc.vector.tensor_scalar_add(out=hv0a, in0=hv0p, scalar1=bia[:, 1:2])

                # -------- software-pipelined per-pair stages --------------------
                st = [dict() for _ in range(PAIRS)]

                def edge_pass_blk(bl, dst, srcs, gelu_bias, hu_m=None):
                    lo0, blen = BLOCKS[bl]
                    ps = pbig.tile([128, PBLK], F32, name="ps", tag="ps_b")
                    for s in range(blen // SUB):
                        lo = lo0 + s * SUB
                        out_sl = ps[:, s * SUB : (s + 1) * SUB]
                        for k, (lhsT, rhs_fn) in enumerate(srcs):
                            nc.tensor.matmul(
                                out_sl,
                                lhsT,
                                rhs_fn(lo),
                                start=(k == 0),
                                stop=(k == len(srcs) - 1),
                            )
                    if hu_m is not None:
                        psv = ps[:, 0:blen].rearrange("p (j i) -> p j i", i=N)
                        nc.vector.tensor_add(
                            out=psv, in0=psv,
                            in1=hu_m.unsqueeze(1).broadcast_to(
                                [128, blen // N, N]
                            ),
                        )
                    nc.scalar.activation(
                        out=dst[:, 0:blen], in_=ps[:, 0:blen], func=AF.Gelu,
                        bias=gelu_bias, scale=1.0,
                    )

                def agg_layer(src_g, wagg, bias_col, acc_name):
                    acc = small.tile([128, N], F32, name=acc_name, tag="sm")
                    accd = small.tile([128, N], F32R, name=acc_name + "d", tag="sm")
                    mjb = PBLK // N
                    for bl, (lo0, blen) in enumerate(BLOCKS):
                        gb = gblk.tile([128, PBLK], EDT, name="gb", tag="gb")
                        edge_pass_blk(
                            bl, gb,
                            [(wagg, lambda lo: src_g[:, lo : lo + SUB])],
                            bias_col,
                        )
                        jlo = lo0 // N
                        jb = blen // N
                        gv = gb[:, 0:blen].rearrange("p (j i) -> p j i", i=N)
                        if EDGE_BF16:
                            f1 = fold.tile([128, mjb, 32], BF16, name="f1",
                                           tag="f1")
                            nc.vector.tensor_add(
                                out=f1[:, 0:jb, :], in0=gv[:, :, 0:32],
                                in1=gv[:, :, 32:64],
                            )
                            f2 = fold.tile([128, mjb, 16], BF16, name="f2",
                                           tag="f2")
                            nc.vector.tensor_add(
                                out=f2[:, 0:jb, :], in0=f1[:, 0:jb, 0:16],
                                in1=f1[:, 0:jb, 16:32],
                            )
                            nc.vector.reduce_sum(
                                out=acc[:, jlo : jlo + jb], in_=f2[:, 0:jb, :],
                                axis=AX.X,
                            )
                        else:
                            nc.vector.reduce_sum(
                                out=acc[:, jlo : jlo + jb], in_=gv, axis=AX.X,
                            )
                        diag = gb[:, jlo : jlo + (jb - 1) * (N + 1) + 1 : N + 1]
                        nc.vector.tensor_sub(
                            out=accd[:, jlo : jlo + jb],
                            in0=acc[:, jlo : jlo + jb],
                            in1=diag,
                        )
                    return accd

                def stage1(p):
                    s = st[p]
                    hv0 = hv0a[:, p * N : (p + 1) * N]
                    s["hv0"] = hv0
                    if HU_DVE:
                        # hu_m[m, i] = (Wtop0_bd.T @ hv0)[m, i] for DVE bcast-add
                        hu0p = psmall.tile([128, 64], F32, name="hu0p", tag="ps_s")
                        nc.tensor.matmul(hu0p, wt0, hv0)
                        hu0 = small.tile([128, 64], F32, name="hu0", tag="sm")
                        nc.vector.tensor_copy(out=hu0, in_=hu0p)
                    else:
                        hu0p = psmall.tile([64, 128], F32, name="hu0p", tag="ps_s")
                        nc.tensor.matmul(hu0p, hv0, wt0)
                        hu0 = small.tile([64, 128], F32R, name="hu0", tag="sm")
                        nc.vector.tensor_copy(out=hu0, in_=hu0p)
                    if EDGE_BF16:
                        ein = big.tile([8, E], F32R, name="ein", tag="ein",
                                       bufs=3)
                    else:
                        ein = big.tile([8, E], F32R, name="ein", tag="stream")
                    nc.gpsimd.dma_start(
                        out=ein[0:4, :], in_=dre_pk[16 * p : 16 * p + 16, :]
                    )
                    nc.gpsimd.dma_start(
                        out=ein[4:6, :], in_=r2_pk[16 * p : 16 * p + 16, :]
                    )
                    nc.gpsimd.dma_start(
                        out=ein[6:8, :], in_=rr_pk[16 * p : 16 * p + 16, :]
                    )
                    g1 = big.tile([128, E], EDT, name="g1", tag="stream")
                    s["g1"] = g1
                    for bl, (lo0, blen) in enumerate(BLOCKS):
                        geb = gblk.tile([128, PBLK], EDT, name="geb", tag="gb")
                        edge_pass_blk(
                            bl, geb,
                            [(wein, lambda lo: ein[:, lo : lo + SUB])],
                            bia[:, 2:3],
                        )
                        if HU_DVE:
                            edge_pass_blk(
                                bl, g1[:, lo0 : lo0 + blen],
                                [(wc_e, lambda lo: geb[:, lo - lo0 : lo - lo0 + SUB])],
                                bia[:, 3:4], hu_m=hu0,
                            )
                        else:
                            edge_pass_blk(
                                bl, g1[:, lo0 : lo0 + blen],
                                [
                                    (wc_e, lambda lo: geb[:, lo - lo0 : lo - lo0 + SUB]),
                                    (hu0, lambda lo: eye_bc),
                                ],
                                bia[:, 3:4],
                            )

                def stage2(p):
                    s = st[p]
                    accd0 = agg_layer(s["g1"], wa0_e, bia[:, 4:5], "acc0")
                    u0p = psmall.tile([128, N], F32, name="u0p", tag="ps_s")
                    nc.tensor.matmul(u0p, wg0, accd0)
                    hv1 = small.tile([128, N], F32R, name="hv1", tag="sm")
                    nc.vector.scalar_tensor_tensor(
                        out=hv1, in0=u0p, scalar=bia[:, 7:8], in1=s["hv0"],
                        op0=mybir.AluOpType.add, op1=mybir.AluOpType.add,
                    )
                    s["hv1"] = hv1
                    if HU_DVE:
                        hu1p = psmall.tile([128, 64], F32, name="hu1p", tag="ps_s")
                        nc.tensor.matmul(hu1p, wt1, hv1)
                        hu1 = small.tile([128, 64], F32, name="hu1", tag="sm")
                        nc.vector.tensor_copy(out=hu1, in_=hu1p)
                    else:
                        hu1p = psmall.tile([64, 128], F32, name="hu1p", tag="ps_s")
                        nc.tensor.matmul(hu1p, hv1, wt1)
                        hu1 = small.tile([64, 128], F32R, name="hu1", tag="sm")
                        nc.vector.tensor_copy(out=hu1, in_=hu1p)
                    s["hu1"] = hu1

                def stage3(p):
                    s = st[p]
                    g1 = s["g1"]
                    hu1 = s["hu1"]
                    g3 = big.tile([128, E], EDT, name="g3", tag="stream")
                    s["g3"] = g3
                    for bl, (lo0, blen) in enumerate(BLOCKS):
                        if HU_DVE:
                            edge_pass_blk(
                                bl, g3[:, lo0 : lo0 + blen],
                                [(wb1_e, lambda lo: g1[:, lo : lo + SUB])],
                                bia[:, 5:6], hu_m=hu1,
                            )
                        else:
                            edge_pass_blk(
                                bl, g3[:, lo0 : lo0 + blen],
                                [
                                    (wb1_e, lambda lo: g1[:, lo : lo + SUB]),
                                    (hu1, lambda lo: eye_bc),
                                ],
                                bia[:, 5:6],
                            )

                def stage4(p):
                    s = st[p]
                    accd1 = agg_layer(s["g3"], wa1_e, bia[:, 6:7], "acc1")
                    u1p = psmall.tile([128, N], F32, name="u1p", tag="ps_s")
                    nc.tensor.matmul(u1p, wg1, accd1)
                    hv2 = small.tile([128, N], F32R, name="hv2", tag="sm")
                    nc.vector.scalar_tensor_tensor(
                        out=hv2, in0=u1p, scalar=bia[:, 8:9], in1=s["hv1"],
                        op0=mybir.AluOpType.add, op1=mybir.AluOpType.add,
                    )
                    thp = psmall.tile([128, N], F32, name="thp", tag="ps_s")
                    nc.tensor.matmul(thp, wh1, hv2)
                    th = small.tile([128, N], F32R, name="th", tag="sm")
                    nc.scalar.activation(
                        out=th, in_=thp, func=AF.Tanh, bias=bia[:, 9:10], scale=1.0
                    )
                    dxp = psmall.tile([4, N], F32, name="dxp", tag="ps_s")
                    nc.tensor.matmul(dxp, wh2, th)
                    dx = small.tile([4, N], F32, name="dx", tag="sm")
                    nc.vector.tensor_scalar_add(out=dx, in0=dxp, scalar1=bia[0:4, 10:11])
                    msum = small.tile([4, 1], F32, name="msum", tag="sm1")
                    nc.vector.reduce_sum(out=msum, in_=dx, axis=AX.X)
                    negm = small.tile([4, 1], F32, name="negm", tag="sm1")
                    nc.vector.tensor_scalar_mul(out=negm, in0=msum, scalar1=-1.0 / N)
                    dxf = small.tile([4, N], F32, name="dxf", tag="sm")
                    nc.vector.tensor_scalar_add(out=dxf, in0=dx, scalar1=negm)
                    nc.sync.dma_start(
                        out=out_dx[2 * p].transpose([1, 0]), in_=dxf[0:2, :]
                    )
                    nc.sync.dma_start(
                        out=out_dx[2 * p + 1].transpose([1, 0]), in_=dxf[2:4, :]
                    )

                stages = [stage1, stage2, stage3, stage4]
                order = [int(c) for c in os.environ.get("BACKFLOW_ORDER", "0132")]
                for t in range(PAIRS + len(stages) - 1):
                    for si in order:
                        p = t - si
                        if 0 <= p < PAIRS:
                            stages[si](p)

    patched = _legalize_sync(nc.to_json_bytes())
    nc.to_json_bytes = lambda: patched
    return nc


def _prep_weights(inputs):
    f8 = np.float64
    g = {k: np.asarray(v, dtype=f8) for k, v in inputs.items()}
    inv = 1.0 / (N - 1)

    wtop0 = g["v2e_w1"][0][:H]
    wbot0 = g["v2e_w1"][0][H:]
    wtop1 = g["v2e_w1"][1][:H]
    wbot1 = g["v2e_w1"][1][H:]

    w_c = g["edge_w2"] @ wbot0
    b_p2 = g["edge_b2"] @ wbot0 + g["v2e_b1"][0]
    w_a0 = g["v2e_w2"][0] @ g["e2v_w1"][0]
    b_p3 = g["v2e_b2"][0] @ g["e2v_w1"][0] + g["e2v_b1"][0]
    w_b1 = g["v2e_w2"][0] @ wbot1
    b_p4 = g["v2e_b2"][0] @ wbot1 + g["v2e_b1"][1]
    w_a1 = g["v2e_w2"][1] @ g["e2v_w1"][1]
    b_p5 = g["v2e_b2"][1] @ g["e2v_w1"][1] + g["e2v_b1"][1]
    w_g0 = g["e2v_w2"][0] * inv
    w_g1 = g["e2v_w2"][1] * inv
    sp = float(np.log1p(np.exp(g["scale"][0])))
    w_h2 = g["head_w2"] * sp
    b_h2 = g["head_b2"] * sp

    def bd(w):  # [64,64] -> [128,128] block-diag
        o = np.zeros((128, 128), f8)
        o[:64, :64] = w
        o[64:, 64:] = w
        return o

    def dup(b):  # [64] -> [128]
        return np.concatenate([b, b])

    ws = {}
    wn1 = np.zeros((6, 128), f8)
    wn1[0:3, 0:64] = g["node_w1"]
    wn1[3:6, 64:128] = g["node_w1"]
    ws["w_node1"] = wn1
    ws["w_node2"] = bd(g["node_w2"])
    e1 = g["edge_w1"]
    wein = np.zeros((8, 128), f8)
    wein[0, 0:64] = e1[0]     # dr0 walker a
    wein[1, 0:64] = e1[1]     # dr1 walker a
    wein[2, 64:128] = e1[0]   # dr0 walker b
    wein[3, 64:128] = e1[1]   # dr1 walker b
    wein[4, 0:64] = e1[3]     # r2 walker a
    wein[5, 64:128] = e1[3]   # r2 walker b
    wein[6, 0:64] = e1[2]     # rr walker a
    wein[7, 64:128] = e1[2]   # rr walker b
    ws["w_ein"] = wein
    ws["w_c"] = bd(w_c)
    ws["w_a0"] = bd(w_a0)
    ws["w_b1"] = bd(w_b1)
    ws["w_a1"] = bd(w_a1)
    ws["w_g0"] = bd(w_g0)
    ws["w_g1"] = bd(w_g1)
    ws["w_top0"] = bd(wtop0)
    ws["w_top1"] = bd(wtop1)
    ws["w_h1"] = bd(g["head_w1"])
    wh2 = np.zeros((128, 4), f8)
    wh2[0:64, 0:2] = w_h2
    wh2[64:128, 2:4] = w_h2
    ws["w_h2"] = wh2
    ws["eye64"] = np.eye(64, dtype=f8)

    bias = np.zeros((128, 12), f8)
    bias[:, 0] = dup(g["node_b1"])
    bias[:, 1] = dup(g["node_b2"])
    bias[:, 2] = dup(g["edge_b1"])
    bias[:, 3] = dup(b_p2)
    bias[:, 4] = dup(b_p3)
    bias[:, 5] = dup(b_p4)
    bias[:, 6] = dup(b_p5)
    bias[:, 7] = dup(g["e2v_b2"][0])
    bias[:, 8] = dup(g["e2v_b2"][1])
    bias[:, 9] = dup(g["head_b1"])
    bias[0:4, 10] = [b_h2[0], b_h2[1], b_h2[0], b_h2[1]]
    ws["biases"] = bias
    out = {k: np.ascontiguousarray(v, dtype=np.float32) for k, v in ws.items()}

    def pack_bf16(a):  # [128,128] f32 -> [128,64] f32 holding 2 bf16 each
        import ml_dtypes
        u16 = np.ascontiguousarray(
            np.asarray(a, np.float32).astype(ml_dtypes.bfloat16)
        ).view(np.uint16)
        u32 = u16[:, 0::2].astype(np.uint32) | (
            u16[:, 1::2].astype(np.uint32) << 16
        )
        return np.ascontiguousarray(u32).view(np.float32)

    for k in ("w_c", "w_a0", "w_b1", "w_a1"):
        out[k + "_bf"] = pack_bf16(out[k])
    return out


def _pack_consts(ws, xt, st_):
    """Assemble the [128, 2352] wpack array (see _build column map)."""
    wp = np.zeros((128, 2352), np.float32)

    def put(col, arr, rows=128):
        a = np.asarray(arr, np.float32)
        wp[: a.shape[0], col : col + a.shape[1]] = a

    put(0, ws["w_node1"])
    put(128, ws["w_node2"])
    put(256, ws["w_ein"])
    put(384, ws["w_c"])
    put(512, ws["w_a0"])
    put(640, ws["w_b1"])
    put(768, ws["w_a1"])
    put(896, ws["w_g0"])
    put(1024, ws["w_g1"])
    put(1152, ws["w_top0"])
    put(1280, ws["w_top1"])
    put(1408, ws["w_h1"])
    put(1536, ws["w_h2"])
    put(1540, ws["eye64"])
    put(1604, ws["biases"])
    BCl, Nl = xt.shape[0], xt.shape[2]
    put(1616, xt.reshape(BCl, 2, 8, 8).transpose(0, 2, 1, 3).reshape(128, 16))
    put(1632, np.repeat(xt.reshape(BCl, 1, 2 * Nl), 8, axis=1).reshape(128, 2 * Nl))
    put(1760, xt.reshape(128, 16))
    put(1776, np.repeat(xt.reshape(BCl * 2, 1, Nl), 4, axis=1).reshape(128, Nl))
    nin = np.concatenate([xt, st_], axis=1).reshape(PAIRS, 6, Nl)
    put(1840, nin.transpose(1, 0, 2).reshape(6, 8 * Nl))
    return wp


def _pack_bf_consts(ws):
    wb = np.zeros((128, 256), np.float32)
    wb[:, 0:64] = ws["w_c_bf"]
    wb[:, 64:128] = ws["w_a0_bf"]
    wb[:, 128:192] = ws["w_b1_bf"]
    wb[:, 192:256] = ws["w_a1_bf"]
    return wb


def kernel(**inputs) -> np.ndarray:
    x = np.asarray(inputs["x"], dtype=np.float32)       # [B, N, D]
    spin = np.asarray(inputs["spin"], dtype=np.float32) # [B, N, 1]
    ws = _prep_weights(inputs)

    if "nc" not in _BUILT:
        _BUILT["nc"] = _build()
    nc = _BUILT["nc"]

    in_maps = []
    for c in range(NCORES):
        xc = x[c * BC : (c + 1) * BC]                     # [16, N, 2]
        sc = spin[c * BC : (c + 1) * BC]                  # [16, N, 1]
        xt = np.ascontiguousarray(xc.transpose(0, 2, 1))  # [16, 2, N]
        st = np.ascontiguousarray(sc.transpose(0, 2, 1))  # [16, 1, N]
        in_maps.append({"wpack": _pack_consts(ws, xt, st),
                        "wbpack": _pack_bf_consts(ws)})

    res = run_bass_kernel_spmd(
        nc,
        in_maps,
        core_ids=list(range(NCORES)),
        trace=os.environ.get("BACKFLOW_TRACE", "0") == "1",
    )
    kernel.last_results = res
    out = np.concatenate([r["out_dx"] for r in res.results], axis=0)
    return out.astype(np.float32)

